# revision 1
# baseline (speedup 1.0000x reference)
"""Trainium2 Bass kernel for nn_AttentionModule_7146825580577.

Strategy (see spec sharding_hint): pure data parallel over the batch dim
(8192 rows -> 1024 rows per core, 8 cores), weights replicated.

Device math (per core), in feature-transposed layout (features on SBUF
partitions, batch on the free dim), fp32 data with float32r matmuls:

  - All LayerNorms whose input is an affine function of a previous
    activation use host-side column-centered weights, so mean(y) == 0 by
    construction and only sum(y^2) is needed on device (computed by a
    ones-vector matmul on the PE, reduced over partitions).
  - seq_len==1 MHA reduces to out_proj(v_proj(kv)); both projections are
    fused on the host into a single 512x512 effective matrix. The self-
    attention residual (x + sa(x)) is folded into a single matmul with
    weights I + Wv@Wo.
  - The cross-attention pair average (a+b)/2 is a single concat-matmul.
  - The n2 LayerNorm (after gating) is folded into the fus_W1 matmul:
    gamma scales fold into the weights, the per-sample mean correction is
    a rank-1 matmul term, betas fold into the bias.
  - 1/sqrt(var+eps) is computed on the vector engine with the int32 bit
    trick + Newton-Raphson iterations, on PE-transposed [128, k] stat
    tiles so each op touches only a tiny free dim.
  - Input hidden states / logits are transposed on the host (numpy) so no
    on-device transposes are needed; the output is produced transposed
    and transposed back on the host.
"""
import os
import sys

sys.path.insert(0, "/opt/trn_rl_repo")

import numpy as np

import concourse.bass as bass
import concourse.tile as tile
from concourse import bacc, mybir
from concourse.bass import ts
from concourse.bass_utils import run_bass_kernel_spmd
from concourse.masks import make_identity

D = 512
HID = 1024
B = 8192
NCORES = 8
BL = B // NCORES          # rows per core
NBT = BL // D             # batch tiles per core (2)
EPS = 1e-5
MAGIC = 0x5F3759DF
F32 = mybir.dt.float32
I32 = mybir.dt.int32
FS = [10, 6, 15]          # logit dims per stream
NR_ITERS = int(os.environ.get("KERNEL_NR_ITERS", "2"))
MM_DT = {
    "f32r": mybir.dt.float32r,
    "f32": mybir.dt.float32,
}[os.environ.get("KERNEL_MM_DTYPE", "f32r")]

F64 = np.float64


# --------------------------------------------------------------------------
# Host-side weight folding
# --------------------------------------------------------------------------

def _center_cols(W, b):
    W = np.asarray(W, F64)
    b = np.asarray(b, F64)
    return W - W.mean(axis=1, keepdims=True), b - b.mean()


def fold_weights(inp):
    g = lambda k: np.asarray(inp[k], dtype=F64)
    out = {}

    w_hp, b_hp = [], []
    for s in range(3):
        W, b = _center_cols(g("hp_W")[s], g("hp_b")[s])
        w_hp.append(W)
        b_hp.append(b)
    out["w_hp"] = np.stack(w_hp)
    out["b_hp"] = np.stack(b_hp)
    out["g_hp"], out["be_hp"] = g("hp_g"), g("hp_be")

    mhaW, mhab = g("mha_in_W"), g("mha_in_b")
    moW, mob = g("mha_out_W"), g("mha_out_b")
    Wv0, bv0 = mhaW[0][:, 2 * D:], mhab[0][2 * D:]
    Wr, br = _center_cols(np.eye(D) + Wv0 @ moW[0], bv0 @ moW[0] + mob[0])
    out["w_r"], out["b_r"] = Wr, br
    out["g_n1"], out["be_n1"] = g("n1_g"), g("n1_be")

    Wj, bj = [None] * 4, [None] * 4
    for j in (1, 2, 3):
        Wv, bv = mhaW[j][:, 2 * D:], mhab[j][2 * D:]
        Wj[j] = Wv @ moW[j]
        bj[j] = bv @ moW[j] + mob[j]
    # m_verb uses (inst_e @ W1, target_e @ W2); m_inst (verb @ W1, target @ W3);
    # m_target (verb @ W2, inst @ W3)
    mods = [(1, 2), (1, 3), (2, 3)]
    out["m_streams"] = [(1, 2), (0, 2), (0, 1)]
    w_m, b_m = [], []
    for s in range(3):
        ja, jb = mods[s]
        w_m.append(np.concatenate([0.5 * Wj[ja], 0.5 * Wj[jb]], axis=0))
        b_m.append(0.5 * (bj[ja] + bj[jb]))
    out["w_m"] = np.stack(w_m)
    out["b_m"] = np.stack(b_m)

    out["w_g"] = g("gate_W")
    out["b_g"] = g("gate_b")

    w_lp, b_lp = [], []
    for s, key in enumerate(["verb", "inst", "target"]):
        W, b = _center_cols(g(f"lp_W_{key}"), g(f"lp_b_{key}"))
        w_lp.append(W)
        b_lp.append(b)
    out["w_lp"] = w_lp
    out["b_lp"] = np.stack(b_lp)
    out["g_lp"], out["be_lp"] = g("lp_g"), g("lp_be")

    W1 = g("fus_W1")
    g2, be2 = g("n2_g"), g("n2_be")
    A1, negc = [], []
    bias_total = g("fus_b1").copy()
    for s in range(3):
        blk = W1[s * D:(s + 1) * D]
        A = g2[s][:, None] * blk
        c = blk.T @ g2[s]
        A1.append(A - A.mean(axis=1, keepdims=True))
        negc.append(-(c - c.mean()))
        bias_total += be2[s] @ blk
    L1 = []
    for s in range(3):
        off = 3 * D + s * (D // 2)
        blk = W1[off: off + D // 2]
        L1.append(blk - blk.mean(axis=1, keepdims=True))
    out["w_f1"] = np.stack(A1)
    out["negc_f1"] = np.stack(negc)
    out["w_f1l"] = np.stack(L1)
    out["b_f1"] = bias_total - bias_total.mean()
    out["g_f1"], out["be_f1"] = g("fus_g1"), g("fus_ge1")

    W2c, b2c = _center_cols(g("fus_W2"), g("fus_b2"))
    out["w_f2"], out["b_f2"] = W2c, b2c
    out["g_f2"], out["be_f2"] = g("fus_g2"), g("fus_ge2")
    return out


def _vec_pp(v, nk):
    """[.., nk*128] feature vector -> ACT per-partition layout [.., 128, nk]."""
    v = np.asarray(v, np.float32)
    return np.ascontiguousarray(v.reshape(v.shape[:-1] + (nk, 128)).swapaxes(-1, -2))


def device_arrays(fw):
    """Folded weights -> dict of fp32 arrays matching the DRAM tensor decls."""
    f32 = lambda v: np.ascontiguousarray(np.asarray(v, np.float32))
    dev = {}
    dev["w_hp"] = f32(fw["w_hp"].reshape(3, 8, 128, 512))
    dev["b_hp"] = _vec_pp(fw["b_hp"], 4)
    dev["w_r"] = f32(fw["w_r"].reshape(4, 128, 512))
    dev["b_r"] = _vec_pp(fw["b_r"], 4)
    dev["w_m"] = f32(fw["w_m"].reshape(3, 8, 128, 512))
    dev["b_m"] = _vec_pp(fw["b_m"], 4)
    dev["w_g"] = f32(fw["w_g"].reshape(3, 8, 128, 512))
    for s in range(3):
        dev[f"w_lp{s}"] = f32(fw["w_lp"][s])
    dev["b_lp"] = _vec_pp(fw["b_lp"], 2)
    dev["w_f1"] = f32(fw["w_f1"].reshape(3, 4, 128, 512))
    dev["w_f1l"] = f32(fw["w_f1l"].reshape(3, 2, 128, 512))
    dev["negc_f1"] = f32(fw["negc_f1"][None])
    dev["b_f1"] = _vec_pp(fw["b_f1"], 4)
    dev["w_f2"] = f32(fw["w_f2"].reshape(4, 128, 512))
    dev["b_f2"] = _vec_pp(fw["b_f2"], 4)
    for name in ("g_hp", "be_hp", "g_n1", "be_n1", "b_g"):
        dev[name] = _vec_pp(fw[name], 4)
    dev["g_lp"] = _vec_pp(fw["g_lp"], 2)
    dev["be_lp"] = _vec_pp(fw["be_lp"], 2)
    for name in ("g_f1", "be_f1", "g_f2", "be_f2"):
        dev[name] = _vec_pp(fw[name], 4)
    dev["ones_row"] = np.ones((1, 128), np.float32)
    dev["ones_col"] = np.ones((128, 1), np.float32)
    return dev


# --------------------------------------------------------------------------
# Device program
# --------------------------------------------------------------------------

class _Emit:
    def __init__(self, tc, io):
        self.tc = tc
        self.nc = tc.nc
        self.io = io
        self.ctx = None
        self.flip = 0

    def alt(self):
        """Alternate DVE / ACT for plain copies and squares."""
        self.flip ^= 1
        return self.flip

    def copy(self, out, in_, bias=None):
        """PSUM -> SBUF eviction, optionally adding a per-partition [128,1]
        bias column (the layer bias in transposed layout)."""
        nc = self.nc
        if self.alt():
            if bias is None:
                nc.vector.tensor_copy(out, in_)
            else:
                nc.vector.tensor_scalar_add(out, in_, bias)
        else:
            if bias is None:
                nc.scalar.activation(out, in_,
                                     mybir.ActivationFunctionType.Copy)
            else:
                nc.scalar.activation(out, in_,
                                     mybir.ActivationFunctionType.Identity,
                                     bias=bias)

    def square(self, out, in_sbuf, in_psum):
        """Square either from the evicted SBUF copy (DVE) or PSUM (ACT)."""
        nc = self.nc
        if self.alt():
            nc.vector.tensor_mul(out, in_sbuf, in_sbuf)
        else:
            nc.scalar.activation(out, in_psum,
                                 mybir.ActivationFunctionType.Square)


MF = MM_DT  # dtype of every tensor consumed by a matmul


def _rd(ap):
    return ap


DEBUG = bool(os.environ.get("KERNEL_DEBUG"))


def emit_program(tc, io):
    nc = tc.nc

    def dbg(name, tile_ap):
        if DEBUG and name in io:
            nc.sync.dma_start(io[name], tile_ap)
    from contextlib import ExitStack
    ctx = ExitStack()
    em = _Emit(tc, io)
    ACT = mybir.ActivationFunctionType

    # ---------------- pools ----------------
    P = lambda name, bufs, space="SBUF": ctx.enter_context(
        tc.tile_pool(name=name, bufs=bufs, space=space))
    const = P("const", 1)
    wpool = P("wchunk", 3)
    xpool = P("xchunk", 2)
    evp = P("ev", 10)
    sqp = P("sq", 2)
    zp = P("z", 2)
    yhp = P("yh", 1)
    ep = P("e", 3)
    mp = P("m", 2)
    sgp = P("sg", 1)
    qp = P("q", 1)
    tp = P("t", 2)
    ztp = P("zt", 3)
    lp_ = P("l", 3)
    hp_ = P("h", 1)
    op_ = P("o", 1)
    stp = P("stats_sb", 9)
    bcp = P("bc_sb", 2)
    ltp = P("lt", 1)
    mm_ps = P("mm_ps", 4, "PSUM")
    st_ps = P("st_ps", 4, "PSUM")

    # ---------------- constants / resident weights ----------------
    ident = const.tile([128, 128], F32)
    make_identity(nc, ident)
    ones_row = const.tile([1, 128], MF)
    nc.sync.dma_start(ones_row[:], io["ones_row"])
    ones_col = const.tile([128, 1], MF)
    nc.sync.dma_start(ones_col[:], io["ones_col"])

    def load(name, shape, rearr=None, dtype=F32):
        t = const.tile(shape, dtype, name=name)
        src = io[name]
        if rearr:
            src = src.rearrange(rearr)
        nc.sync.dma_start(t[:], src)
        return t

    b_hp = load("b_hp", [128, 3, 4], "s p c -> p s c")
    b_r = load("b_r", [128, 4])
    b_m = load("b_m", [128, 3, 4], "s p c -> p s c")
    b_lp = load("b_lp", [128, 3, 2], "s p c -> p s c")
    negc = load("negc_f1", [1, 3, 512], dtype=MF)
    b_f1 = load("b_f1", [128, 4])
    b_f2 = load("b_f2", [128, 4])
    g_hp = load("g_hp", [128, 3, 4], "s p c -> p s c")
    be_hp = load("be_hp", [128, 3, 4], "s p c -> p s c")
    g_n1 = load("g_n1", [128, 3, 4], "s p c -> p s c")
    be_n1 = load("be_n1", [128, 3, 4], "s p c -> p s c")
    b_g = load("b_g", [128, 3, 4], "s p c -> p s c")
    g_lp = load("g_lp", [128, 3, 2], "s p c -> p s c")
    be_lp = load("be_lp", [128, 3, 2], "s p c -> p s c")
    g_f1 = load("g_f1", [128, 4])
    be_f1 = load("be_f1", [128, 4])
    g_f2 = load("g_f2", [128, 4])
    be_f2 = load("be_f2", [128, 4])
    w_lp = [load(f"w_lp{s}", [FS[s], 256], dtype=MF) for s in range(3)]

    # ---------------- helpers ----------------
    def emit_istd(v_sb, k):
        """v_sb: [k,512] sbuf fp32 variances (+eps already added).
        Returns list of k istd row tiles [1,512] (MF), via PE-transposed
        Newton-Raphson rsqrt (int32 magic seed)."""
        vT = st_ps.tile([128, 4 * k], F32, name="vT", tag="stat_ps")
        for c in range(4):
            nc.tensor.transpose(vT[:, c * k:(c + 1) * k],
                                v_sb[0:k, ts(c, 128)], ident[0:k, 0:k])
        y = stp.tile([128, 4 * k], F32, name="nr_y", tag="ssb")
        t = stp.tile([128, 4 * k], F32, name="nr_t", tag="ssb")
        nc.vector.tensor_scalar(y[:].bitcast(I32), vT[:].bitcast(I32),
                                1, None, mybir.AluOpType.logical_shift_right)
        nc.vector.tensor_scalar(y[:].bitcast(I32), y[:].bitcast(I32),
                                -1, MAGIC, mybir.AluOpType.mult,
                                mybir.AluOpType.add)
        for _ in range(NR_ITERS):
            nc.vector.tensor_mul(t[:], y[:], y[:])
            nc.vector.tensor_mul(t[:], t[:], vT[:])
            nc.vector.tensor_scalar(t[:], t[:], -0.5, 1.5,
                                    mybir.AluOpType.mult, mybir.AluOpType.add)
            nc.vector.tensor_mul(y[:], y[:], t[:])
        rows = []
        for s in range(k):
            rT = st_ps.tile([1, 512], F32, name="rT", tag="stat_ps")
            for c in range(4):
                nc.tensor.transpose(rT[0:1, ts(c, 128)],
                                    y[:, c * k + s:c * k + s + 1], ident)
            istd = stp.tile([1, 512], MF, name="istd", tag="ssb")
            nc.vector.tensor_copy(istd[:], rT[:])
            rows.append(istd)
        return rows

    def bcast(row_ap):
        """[1,512] sbuf row -> [128,512] sbuf tile via GPSIMD."""
        bc = bcp.tile([128, 512], MF, name="bc")
        nc.gpsimd.partition_broadcast(bc[:], row_ap)
        return bc

    def emit_ln(ps_list, bias_cols=None):
        """Evict psum chunks to SBUF (adding the layer bias per partition)
        and accumulate sum(y^2) into a [1,512] psum row."""
        nch = len(ps_list)
        ev = []
        for c, ps in enumerate(ps_list):
            e = evp.tile([128, 512], F32, name="ev")
            em.copy(e[:], ps[:], None if bias_cols is None else bias_cols[c])
            ev.append(e)
        st = st_ps.tile([1, 512], F32, name="st", tag="stat_ps")
        for c in range(nch):
            sq = sqp.tile([128, 512], MF, name="sq")
            em.square(sq[:], ev[c][:], ev[c][:])
            nc.tensor.matmul(st[:], ones_col[:], sq[:],
                             start=(c == 0), stop=(c == nch - 1))
        return ev, st

    def ln_finish(ev, st, gam, bet, func, out_tile, dim=D):
        v = stp.tile([1, 512], F32, name="v", tag="ssb")
        nc.vector.tensor_scalar(v[0:1, :], st[:], 1.0 / dim, EPS,
                                mybir.AluOpType.mult, mybir.AluOpType.add)
        istd = emit_istd(v, 1)[0]
        bc = bcast(istd[:])
        for c, e in enumerate(ev):
            z = zp.tile([128, 512], F32, name="z")
            nc.vector.tensor_mul(z[:], e[:], bc[:])
            nc.scalar.activation(out_tile[:, c, :], z[:], func,
                                 bias=bet[:, c:c + 1], scale=gam[:, c:c + 1])

    def mm_group(n_m, srcs, bias_cols):
        """Emit an accumulating matmul group. srcs = list of (lhsT_fn, rhs)
        k-chunks; returns (ev, st) after evict+square+stats."""
        ps_list = [mm_ps.tile([128, 512], F32, name="mm") for _ in range(n_m)]
        last = len(srcs) - 1
        for ci, (lhsT_fn, rhs) in enumerate(srcs):
            for m in range(n_m):
                nc.tensor.matmul(ps_list[m][:], lhsT_fn(m), rhs,
                                 start=(ci == 0), stop=(ci == last))
        return ps_list

    # ---------------- main ----------------
    pend = []

    def flush(n=None):
        cnt = len(pend) if n is None else n
        for _ in range(cnt):
            if pend:
                pend.pop(0)()

    def wchunk(dram_ap):
        wc = wpool.tile([128, 512], MF, name="wc", tag="wc1")
        nc.sync.dma_start(wc[:], dram_ap)
        return wc

    def wpair(dram_pair_ap):
        """Load two [128,512] k-chunks in one DMA -> [128,2,512] tile."""
        wc = wpool.tile([128, 2, 512], MF, name="wcp", tag="wcp")
        nc.sync.dma_start(wc[:], dram_pair_ap.rearrange("c p n -> p c n"))
        return wc

    def pair_srcs(dram_4d, nk, rhs_fn):
        srcs = []
        for c0 in range(0, nk, 2):
            wc = wpair(dram_4d[c0:c0 + 2])
            for cc in range(2):
                srcs.append((lambda m, wc=wc, cc=cc: wc[:, cc, ts(m, 128)],
                             rhs_fn(c0 + cc)))
        return srcs

    for bt in range(NBT):
        bsl = ts(bt, 512)
        l_tiles = [None] * 3
        e_tiles = [None] * 3
        m_tiles = [None] * 3
        zt_tiles = [None] * 3
        w_rows = [None] * 3
        yh_tiles = [None] * 3

        # ---- lp matmuls (tiny) ----
        def emit_lp_mm(s):
            lt = ltp.tile([FS[s], 512], F32, name="lt", tag="lt")
            nc.sync.dma_start(lt[:], io[f"lT{s}"][:, bsl])
            lsg = ltp.tile([FS[s], 512], MF, name="lsg", tag="lsg")
            nc.scalar.activation(lsg[:], lt[:], ACT.Sigmoid)
            ps_list = [mm_ps.tile([128, 512], F32, name="mm") for _ in range(2)]
            for m in range(2):
                nc.tensor.matmul(ps_list[m][:], w_lp[s][:, ts(m, 128)],
                                 lsg[:], start=True, stop=True)
            ev, st = emit_ln(ps_list, [b_lp[:, s, c:c + 1] for c in range(2)])

            def fin(s=s, ev=ev, st=st):
                l_sb = lp_.tile([128, 2, 512], MF, name="l_sb")
                ln_finish(ev, st, g_lp[:, s], be_lp[:, s], ACT.Gelu, l_sb,
                          dim=D // 2)
                l_tiles[s] = l_sb
            pend.append(fin)

        def emit_hp_mm(s):
            xcs = []
            for c0 in range(0, 8, 2):
                xc = xpool.tile([128, 2, 512], MF, name="xc")
                nc.sync.dma_start(
                    xc[:], io[f"xT{s}"][ts(c0 // 2, 256), bsl].rearrange(
                        "(c p) b -> p c b", p=128))
                xcs.append(xc)
            srcs = pair_srcs(io["w_hp"][s], 8,
                             lambda c: xcs[c // 2][:, c % 2, :])
            ps_list = mm_group(4, srcs, None)
            ev, st = emit_ln(ps_list, [b_hp[:, s, c:c + 1] for c in range(4)])

            def fin(s=s, ev=ev, st=st):
                yh = yhp.tile([128, 4, 512], MF, name="yh")
                ln_finish(ev, st, g_hp[:, s], be_hp[:, s], ACT.Gelu, yh)
                yh_tiles[s] = yh
            pend.append(fin)

        def emit_r_mm(s):
            yh = yh_tiles[s]
            srcs = pair_srcs(io["w_r"], 4, lambda c: yh[:, c, :])
            ps_list = mm_group(4, srcs, None)
            ev, st = emit_ln(ps_list, [b_r[:, c:c + 1] for c in range(4)])

            def fin(s=s, ev=ev, st=st):
                e_sb = ep.tile([128, 4, 512], MF, name="e_sb")
                ln_finish(ev, st, g_n1[:, s], be_n1[:, s], ACT.Identity, e_sb)
                e_tiles[s] = e_sb
            pend.append(fin)

        m_streams = [(1, 2), (0, 2), (0, 1)]

        def emit_m_mm(s):
            sa, sb = m_streams[s]
            srcs = pair_srcs(io["w_m"][s], 8,
                             lambda ci: (e_tiles[sa][:, ci, :] if ci < 4
                                         else e_tiles[sb][:, ci - 4, :]))
            ps_list = mm_group(4, srcs, None)
            m_sb = mp.tile([128, 4, 512], MF, name="m_sb")
            for c in range(4):
                em.copy(m_sb[:, c, :], ps_list[c][:], b_m[:, s, c:c + 1])
            m_tiles[s] = m_sb

        def emit_gate_fuse(s):
            srcs = pair_srcs(io["w_g"][s], 8,
                             lambda ci: (e_tiles[s][:, ci, :] if ci < 4
                                         else m_tiles[s][:, ci - 4, :]))
            ps_list = mm_group(4, srcs, None)
            t_sb = tp.tile([128, 4, 512], MF, name="t_sb")
            for c in range(4):
                sg = sgp.tile([128, 512], F32, name="sg")
                nc.scalar.activation(sg[:], ps_list[c][:], ACT.Sigmoid,
                                     bias=b_g[:, s, c:c + 1])
                q = qp.tile([128, 512], F32, name="q")
                nc.vector.tensor_mul(q[:], sg[:], m_tiles[s][:, c, :])
                nc.vector.tensor_add(t_sb[:, c, :], e_tiles[s][:, c, :], q[:])
            st_sum = st_ps.tile([1, 512], F32, name="st_sum", tag="stat_ps")
            st_sq = st_ps.tile([1, 512], F32, name="st_sq", tag="stat_ps")
            for c in range(4):
                nc.tensor.matmul(st_sum[:], ones_col[:], t_sb[:, c, :],
                                 start=(c == 0), stop=(c == 3))
            for c in range(4):
                sq = sqp.tile([128, 512], MF, name="sq")
                em.square(sq[:], t_sb[:, c, :], t_sb[:, c, :])
                nc.tensor.matmul(st_sq[:], ones_col[:], sq[:],
                                 start=(c == 0), stop=(c == 3))

            def fin(s=s, t_sb=t_sb, st_sum=st_sum, st_sq=st_sq):
                mu = stp.tile([1, 512], F32, name="mu", tag="ssb")
                nc.vector.tensor_scalar_mul(mu[:], st_sum[:], 1.0 / D)
                ev2 = stp.tile([1, 512], F32, name="ev2", tag="ssb")
                nc.vector.tensor_scalar(ev2[:], st_sq[:], 1.0 / D, EPS,
                                        mybir.AluOpType.mult,
                                        mybir.AluOpType.add)
                v = stp.tile([1, 512], F32, name="v", tag="ssb")
                nc.vector.tensor_mul(v[:], mu[:], mu[:])
                nc.vector.tensor_sub(v[:], ev2[:], v[:])
                istd = emit_istd(v, 1)[0]
                w_row = stp.tile([1, 512], MF, name="w_row", tag="ssb")
                nc.vector.tensor_mul(w_row[:], mu[:], istd[:])
                w_rows[s] = w_row
                bc = bcast(istd[:])
                zt = ztp.tile([128, 4, 512], MF, name="zt")
                for c in range(4):
                    nc.vector.tensor_mul(zt[:, c, :], t_sb[:, c, :], bc[:])
                zt_tiles[s] = zt
            pend.append(fin)

        # ---------- emission schedule (software pipelined) ----------
        emit_lp_mm(0)
        emit_lp_mm(1)
        emit_lp_mm(2)
        emit_hp_mm(0)          # hp0 matmuls cover lp NR chains
        flush(2)               # lp0, lp1 fins
        emit_hp_mm(1)
        flush(2)               # lp2 fin + hp0 fin (covered by hp1 matmuls)
        emit_r_mm(0)
        flush(1)               # hp1 fin (covered by r'0/hp1 matmuls)
        emit_hp_mm(2)
        emit_r_mm(1)
        flush(1)               # n1_0 fin -> e0
        flush(1)               # hp2 fin -> yh2
        emit_r_mm(2)
        flush(1)               # n1_1 fin -> e1
        emit_m_mm(2)           # m_target needs e0,e1
        flush(1)               # n1_2 fin -> e2
        emit_gate_fuse(2)
        emit_m_mm(1)           # m_inst needs e0,e2
        flush(1)               # n2_2 fin -> zt2 (covered by m1 matmuls)
        emit_gate_fuse(1)
        emit_m_mm(0)           # m_verb needs e1,e2
        flush(1)               # n2_1 fin -> zt1 (covered by m0 matmuls)
        emit_gate_fuse(0)
        flush(1)               # n2_0 fin -> zt0 (covered by fus1 l/zt2/zt1)

        # ---- fus1: order k-chunks so zt0 (finished last) is consumed last
        srcs = []
        for s in range(3):
            srcs += pair_srcs(io["w_f1l"][s], 2,
                              lambda c, s=s: l_tiles[s][:, c, :])
        for s in (2, 1, 0):
            srcs += pair_srcs(io["w_f1"][s], 4,
                              lambda c, s=s: zt_tiles[s][:, c, :])
        for s in (2, 1, 0):
            srcs.append((lambda m, s=s: negc[0:1, s, ts(m, 128)],
                         w_rows[s][:]))
        ps_list = mm_group(4, srcs, None)
        ev, st = emit_ln(ps_list, [b_f1[:, c:c + 1] for c in range(4)])

        def fin_f1(ev=ev, st=st):
            h_sb = hp_.tile([128, 4, 512], MF, name="h_sb")
            ln_finish(ev, st, g_f1, be_f1, ACT.Gelu, h_sb)
            fin_f1.h = h_sb
        pend.append(fin_f1)
        flush(1)

        # ---- fus2
        h_sb = fin_f1.h
        srcs = pair_srcs(io["w_f2"], 4, lambda c: h_sb[:, c, :])
        ps_list = mm_group(4, srcs, None)
        ev, st = emit_ln(ps_list, [b_f2[:, c:c + 1] for c in range(4)])

        def fin_f2(ev=ev, st=st, bsl=bsl):
            o_sb = op_.tile([128, 4, 512], F32, name="o_sb")
            ln_finish(ev, st, g_f2, be_f2, ACT.Identity, o_sb)
            nc.sync.dma_start(
                io["outT"].rearrange("(c p) b -> p c b", p=128)[:, :, bsl],
                o_sb[:])
        pend.append(fin_f2)
        flush(1)

    flush()
    ctx.close()


def build_program():
    nc = bacc.Bacc("TRN2", target_bir_lowering=False, debug=False,
                   num_devices=NCORES)
    io = {}

    def din(name, shape, dtype=F32):
        io[name] = nc.dram_tensor(name, list(shape), dtype,
                                  kind="ExternalInput").ap()

    for s in range(3):
        din(f"xT{s}", (HID, BL), dtype=MM_DT)
        din(f"lT{s}", (FS[s], BL))
    din("w_hp", (3, 8, 128, 512), dtype=MM_DT)
    din("b_hp", (3, 128, 4))
    din("w_r", (4, 128, 512), dtype=MM_DT)
    din("b_r", (128, 4))
    din("w_m", (3, 8, 128, 512), dtype=MM_DT)
    din("b_m", (3, 128, 4))
    din("w_g", (3, 8, 128, 512), dtype=MM_DT)
    for s in range(3):
        din(f"w_lp{s}", (FS[s], 256), dtype=MM_DT)
    din("b_lp", (3, 128, 2))
    din("w_f1", (3, 4, 128, 512), dtype=MM_DT)
    din("w_f1l", (3, 2, 128, 512), dtype=MM_DT)
    din("negc_f1", (1, 3, 512), dtype=MM_DT)
    din("b_f1", (128, 4))
    din("w_f2", (4, 128, 512), dtype=MM_DT)
    din("b_f2", (128, 4))
    for name in ("g_hp", "be_hp", "g_n1", "be_n1", "b_g"):
        din(name, (3, 128, 4))
    for name in ("g_lp", "be_lp"):
        din(name, (3, 128, 2))
    for name in ("g_f1", "be_f1", "g_f2", "be_f2"):
        din(name, (128, 4))
    din("ones_row", (1, 128), dtype=MM_DT)
    din("ones_col", (128, 1), dtype=MM_DT)
    io["outT"] = nc.dram_tensor("outT", [D, BL], F32,
                                kind="ExternalOutput").ap()
    if os.environ.get("KERNEL_DEBUG"):
        for s in range(3):
            for nm, shp in [(f"dbg_istd_hp{s}", [1, 512]),
                            (f"dbg_yh{s}", [128, 4, 512]),
                            (f"dbg_e{s}", [128, 4, 512]),
                            (f"dbg_l{s}", [128, 2, 512]),
                            (f"dbg_m{s}", [128, 4, 512]),
                            (f"dbg_t{s}", [128, 4, 512]),
                            (f"dbg_w{s}", [1, 512])]:
                io[nm] = nc.dram_tensor(nm, shp, F32,
                                        kind="ExternalOutput").ap()
        io["dbg_h"] = nc.dram_tensor("dbg_h", [128, 4, 512], F32,
                                     kind="ExternalOutput").ap()

    with tile.TileContext(nc) as tc:
        emit_program(tc, io)
    nc.compile()
    return nc


def make_in_maps(inputs):
    fw = fold_weights(inputs)
    dev = device_arrays(fw)
    hidden = [np.asarray(inputs["verb_hidden"], np.float32),
              np.asarray(inputs["inst_hidden"], np.float32),
              np.asarray(inputs["target_hidden"], np.float32)]
    logits = [np.asarray(inputs["verb_logits"], np.float32),
              np.asarray(inputs["inst_logits"], np.float32),
              np.asarray(inputs["target_logits"], np.float32)]
    in_maps = []
    for core in range(NCORES):
        rows = slice(core * BL, (core + 1) * BL)
        m = dict(dev)
        for s in range(3):
            m[f"xT{s}"] = np.ascontiguousarray(hidden[s][rows].T)
            m[f"lT{s}"] = np.ascontiguousarray(logits[s][rows].T)
        in_maps.append(m)
    return in_maps


_NC_CACHE = None


def _run(inputs, **spmd_kwargs):
    global _NC_CACHE
    if _NC_CACHE is None:
        _NC_CACHE = build_program()
    nc = _NC_CACHE
    in_maps = make_in_maps(inputs)
    res = run_bass_kernel_spmd(nc, in_maps, list(range(NCORES)),
                               **spmd_kwargs)
    out = np.empty((B, D), dtype=np.float32)
    for core in range(NCORES):
        out[core * BL:(core + 1) * BL] = res.results[core]["outT"].T
    return out, res


def kernel(**inputs) -> np.ndarray:
    return _run(inputs)[0]


def kernel_profiled(inputs, tmpdir=None):
    """Returns (out, BassKernelResults) with an NTFF-based profile."""
    return _run(inputs, trace=True, tmpdir=tmpdir)



# revision 10
# speedup vs baseline: 1.3446x; 1.3446x over previous
"""Trainium2 Bass kernel for nn_AttentionModule_7146825580577.

Strategy (see spec sharding_hint): pure data parallel over the batch dim
(8192 rows -> 1024 rows per core, 8 cores), weights replicated.

Device math (per core), in feature-transposed layout (features on SBUF
partitions, batch on the free dim), fp32 data with float32r matmuls:

  - All LayerNorms whose input is an affine function of a previous
    activation use host-side column-centered weights, so mean(y) == 0 by
    construction and only sum(y^2) is needed on device (computed by a
    ones-vector matmul on the PE, reduced over partitions).
  - seq_len==1 MHA reduces to out_proj(v_proj(kv)); both projections are
    fused on the host into a single 512x512 effective matrix. The self-
    attention residual (x + sa(x)) is folded into a single matmul with
    weights I + Wv@Wo.
  - The cross-attention pair average (a+b)/2 is a single concat-matmul.
  - The n2 LayerNorm (after gating) is folded into the fus_W1 matmul:
    gamma scales fold into the weights, the per-sample mean correction is
    a rank-1 matmul term, betas fold into the bias.
  - 1/sqrt(var+eps) is computed on the vector engine with the int32 bit
    trick + Newton-Raphson iterations, on PE-transposed [128, k] stat
    tiles so each op touches only a tiny free dim.
  - Input hidden states / logits are transposed on the host (numpy) so no
    on-device transposes are needed; the output is produced transposed
    and transposed back on the host.
"""
import os
import sys

sys.path.insert(0, "/opt/trn_rl_repo")

import numpy as np

import concourse.bass as bass
import concourse.tile as tile
from concourse import bacc, mybir
from concourse.bass import ts
from concourse.bass_utils import run_bass_kernel_spmd
from concourse.masks import make_identity

D = 512
HID = 1024
B = 8192
NCORES = 8
BL = B // NCORES          # rows per core
NBT = BL // D             # batch tiles per core (2)
EPS = 1e-5
F32 = mybir.dt.float32
I32 = mybir.dt.int32
FS = [10, 6, 15]          # logit dims per stream
_MM_KEY = os.environ.get("KERNEL_MM_DTYPE", "f16")
MM_DT = {
    "f16": mybir.dt.float16,
    "bf16": mybir.dt.bfloat16,
    "f32r": mybir.dt.float32r,
    "f32": mybir.dt.float32,
}[_MM_KEY]
MM_NP = {
    "f16": np.float16,
    "bf16": np.float32,   # bf16 via ml_dtypes if ever needed; f32 placeholder
    "f32r": np.float32,
    "f32": np.float32,
}[_MM_KEY]

F64 = np.float64


# --------------------------------------------------------------------------
# Host-side weight folding
# --------------------------------------------------------------------------

def _center_cols(W, b):
    W = np.asarray(W, F64)
    b = np.asarray(b, F64)
    return W - W.mean(axis=1, keepdims=True), b - b.mean()


def fold_weights(inp):
    g = lambda k: np.asarray(inp[k], dtype=F64)
    out = {}

    w_hp, b_hp = [], []
    for s in range(3):
        W, b = _center_cols(g("hp_W")[s], g("hp_b")[s])
        w_hp.append(W)
        b_hp.append(b)
    out["w_hp"] = np.stack(w_hp)
    out["b_hp"] = np.stack(b_hp)
    out["g_hp"], out["be_hp"] = g("hp_g"), g("hp_be")

    mhaW, mhab = g("mha_in_W"), g("mha_in_b")
    moW, mob = g("mha_out_W"), g("mha_out_b")
    Wv0, bv0 = mhaW[0][:, 2 * D:], mhab[0][2 * D:]
    Wr, br = _center_cols(np.eye(D) + Wv0 @ moW[0], bv0 @ moW[0] + mob[0])
    out["w_r"], out["b_r"] = Wr, br
    out["g_n1"], out["be_n1"] = g("n1_g"), g("n1_be")

    Wj, bj = [None] * 4, [None] * 4
    for j in (1, 2, 3):
        Wv, bv = mhaW[j][:, 2 * D:], mhab[j][2 * D:]
        Wj[j] = Wv @ moW[j]
        bj[j] = bv @ moW[j] + mob[j]
    # m_verb uses (inst_e @ W1, target_e @ W2); m_inst (verb @ W1, target @ W3);
    # m_target (verb @ W2, inst @ W3)
    mods = [(1, 2), (1, 3), (2, 3)]
    out["m_streams"] = [(1, 2), (0, 2), (0, 1)]
    w_m, b_m = [], []
    for s in range(3):
        ja, jb = mods[s]
        w_m.append(np.concatenate([0.5 * Wj[ja], 0.5 * Wj[jb]], axis=0))
        b_m.append(0.5 * (bj[ja] + bj[jb]))
    out["w_m"] = np.stack(w_m)
    out["b_m"] = np.stack(b_m)

    out["w_g"] = g("gate_W")
    out["b_g"] = g("gate_b")

    w_lp, b_lp = [], []
    for s, key in enumerate(["verb", "inst", "target"]):
        W, b = _center_cols(g(f"lp_W_{key}"), g(f"lp_b_{key}"))
        w_lp.append(W)
        b_lp.append(b)
    out["w_lp"] = w_lp
    out["b_lp"] = np.stack(b_lp)
    out["g_lp"], out["be_lp"] = g("lp_g"), g("lp_be")

    W1 = g("fus_W1")
    g2, be2 = g("n2_g"), g("n2_be")
    A1, negc = [], []
    bias_total = g("fus_b1").copy()
    for s in range(3):
        blk = W1[s * D:(s + 1) * D]
        A = g2[s][:, None] * blk
        c = blk.T @ g2[s]
        A1.append(A - A.mean(axis=1, keepdims=True))
        negc.append(-(c - c.mean()))
        bias_total += be2[s] @ blk
    L1 = []
    for s in range(3):
        off = 3 * D + s * (D // 2)
        blk = W1[off: off + D // 2]
        L1.append(blk - blk.mean(axis=1, keepdims=True))
    out["w_f1"] = np.stack(A1)
    out["negc_f1"] = np.stack(negc)
    out["w_f1l"] = np.stack(L1)
    out["b_f1"] = bias_total - bias_total.mean()
    out["g_f1"], out["be_f1"] = g("fus_g1"), g("fus_ge1")

    W2c, b2c = _center_cols(g("fus_W2"), g("fus_b2"))
    out["w_f2"], out["b_f2"] = W2c, b2c
    out["g_f2"], out["be_f2"] = g("fus_g2"), g("fus_ge2")
    return out


def _vec_pp(v, nk):
    """[.., nk*128] feature vector -> ACT per-partition layout [.., 128, nk]."""
    v = np.asarray(v, np.float32)
    return np.ascontiguousarray(v.reshape(v.shape[:-1] + (nk, 128)).swapaxes(-1, -2))


def device_arrays(fw):
    """Folded weights -> dict of arrays matching the DRAM tensor decls.
    Matmul operands are MM_NP (fp16 by default), vector params fp32."""
    fmm = lambda v: np.ascontiguousarray(np.asarray(v, MM_NP))
    dev = {}
    dev["w_hp"] = fmm(fw["w_hp"].reshape(3, 8, 128, 512))
    dev["b_hp"] = _vec_pp(fw["b_hp"], 4)
    dev["w_r"] = fmm(fw["w_r"].reshape(4, 128, 512))
    dev["b_r"] = _vec_pp(fw["b_r"], 4)
    dev["w_m"] = fmm(fw["w_m"].reshape(3, 8, 128, 512))
    dev["b_m"] = _vec_pp(fw["b_m"], 4)
    dev["w_g"] = fmm(fw["w_g"].reshape(3, 8, 128, 512))
    for s in range(3):
        dev[f"w_lp{s}"] = fmm(fw["w_lp"][s])
    dev["b_lp"] = _vec_pp(fw["b_lp"], 2)
    dev["w_f1"] = fmm(fw["w_f1"].reshape(3, 4, 128, 512))
    dev["w_f1l"] = fmm(fw["w_f1l"].reshape(3, 2, 128, 512))
    dev["negc_f1"] = fmm(fw["negc_f1"][None])
    dev["b_f1"] = _vec_pp(fw["b_f1"], 4)
    dev["w_f2"] = fmm(fw["w_f2"].reshape(4, 128, 512))
    dev["b_f2"] = _vec_pp(fw["b_f2"], 4)
    for name in ("g_hp", "be_hp", "g_n1", "be_n1", "b_g"):
        dev[name] = _vec_pp(fw[name], 4)
    dev["g_lp"] = _vec_pp(fw["g_lp"], 2)
    dev["be_lp"] = _vec_pp(fw["be_lp"], 2)
    for name in ("g_f1", "be_f1", "g_f2", "be_f2"):
        dev[name] = _vec_pp(fw[name], 4)
    dev["ones_col"] = np.ones((128, 1), MM_NP)
    return dev


# --------------------------------------------------------------------------
# Device program
# --------------------------------------------------------------------------

class _Emit:
    def __init__(self, tc, io):
        self.tc = tc
        self.nc = tc.nc
        self.io = io
        self.ctx = None
        self.flip = 0

    def alt(self):
        """Alternate DVE / ACT for plain copies and squares."""
        self.flip ^= 1
        return self.flip

    def copy(self, out, in_, bias=None):
        """PSUM -> SBUF eviction, optionally adding a per-partition [128,1]
        bias column (the layer bias in transposed layout)."""
        nc = self.nc
        if self.alt():
            if bias is None:
                nc.vector.tensor_copy(out, in_)
            else:
                nc.vector.tensor_scalar_add(out, in_, bias)
        else:
            if bias is None:
                nc.scalar.activation(out, in_,
                                     mybir.ActivationFunctionType.Copy)
            else:
                nc.scalar.activation(out, in_,
                                     mybir.ActivationFunctionType.Identity,
                                     bias=bias)

    def square(self, out, in_sbuf, in_psum):
        """Square either from the evicted SBUF copy (DVE) or PSUM (ACT)."""
        nc = self.nc
        if self.alt():
            nc.vector.tensor_mul(out, in_sbuf, in_sbuf)
        else:
            nc.scalar.activation(out, in_psum,
                                 mybir.ActivationFunctionType.Square)


MF = MM_DT  # dtype of every tensor consumed by a matmul


def _rd(ap):
    return ap


DEBUG = bool(os.environ.get("KERNEL_DEBUG"))


def emit_program(tc, io):
    nc = tc.nc

    def dbg(name, tile_ap):
        if DEBUG and name in io:
            nc.sync.dma_start(io[name], tile_ap)
    from contextlib import ExitStack
    ctx = ExitStack()
    em = _Emit(tc, io)
    ACT = mybir.ActivationFunctionType

    # ---------------- pools ----------------
    P = lambda name, bufs, space="SBUF": ctx.enter_context(
        tc.tile_pool(name=name, bufs=bufs, space=space))
    const = P("const", 1)
    wpool = P("wchunk", 3)
    xpool = P("xchunk", 2)
    evp = P("ev", 10)
    sqp = P("sq", 2)
    zp = P("z", 2)
    yhp = P("yh", 1)
    ep = P("e", 3)
    mp = P("m", 2)
    sgp = P("sg", 1)
    qp = P("q", 1)
    tp = P("t", 2)
    ztp = P("zt", 3)
    lp_ = P("l", 3)
    hp_ = P("h", 1)
    op_ = P("o", 1)
    stp = P("stats_sb", 9)
    bcp = P("bc_sb", 2)
    ltp = P("lt", 1)
    mm_ps = P("mm_ps", 6, "PSUM")
    st_ps = P("st_ps", 2, "PSUM")

    # ---------------- constants / resident weights ----------------
    ones_col = const.tile([128, 1], MF)
    nc.sync.dma_start(ones_col[:], io["ones_col"])

    def load(name, shape, rearr=None, dtype=F32):
        t = const.tile(shape, dtype, name=name)
        src = io[name]
        if rearr:
            src = src.rearrange(rearr)
        nc.sync.dma_start(t[:], src)
        return t

    b_hp = load("b_hp", [128, 3, 4], "s p c -> p s c")
    b_r = load("b_r", [128, 4])
    b_m = load("b_m", [128, 3, 4], "s p c -> p s c")
    b_lp = load("b_lp", [128, 3, 2], "s p c -> p s c")
    negc = load("negc_f1", [1, 3, 512], dtype=MF)
    b_f1 = load("b_f1", [128, 4])
    b_f2 = load("b_f2", [128, 4])
    g_hp = load("g_hp", [128, 3, 4], "s p c -> p s c")
    be_hp = load("be_hp", [128, 3, 4], "s p c -> p s c")
    g_n1 = load("g_n1", [128, 3, 4], "s p c -> p s c")
    be_n1 = load("be_n1", [128, 3, 4], "s p c -> p s c")
    b_g = load("b_g", [128, 3, 4], "s p c -> p s c")
    g_lp = load("g_lp", [128, 3, 2], "s p c -> p s c")
    be_lp = load("be_lp", [128, 3, 2], "s p c -> p s c")
    g_f1 = load("g_f1", [128, 4])
    be_f1 = load("be_f1", [128, 4])
    g_f2 = load("g_f2", [128, 4])
    be_f2 = load("be_f2", [128, 4])
    w_lp = [load(f"w_lp{s}", [FS[s], 256], dtype=MF) for s in range(3)]

    # ---------------- helpers ----------------
    def emit_istd(v_sb):
        """v_sb: [1,512] sbuf fp32 variance (+eps already added).
        Returns a [1,512] MF istd row via DVE approx-reciprocal + ACT sqrt
        (rsqrt(v) = sqrt(1/v)); ~18-bit accurate, no PE transposes."""
        r = stp.tile([1, 512], F32, name="recip", tag="ssb")
        nc.vector.reciprocal_approx_fast(out=r[:], in_=v_sb[0:1, :])
        istd = stp.tile([1, 512], MF, name="istd", tag="ssb")
        nc.scalar.activation(istd[:], r[:], ACT.Sqrt)
        return istd

    def bcast(row_ap):
        """[1,512] sbuf row -> [128,512] sbuf tile via GPSIMD."""
        bc = bcp.tile([128, 512], MF, name="bc")
        nc.gpsimd.partition_broadcast(bc[:], row_ap)
        return bc

    def emit_ln(ps_list, bias_cols=None):
        """Evict psum chunks to SBUF (adding the layer bias per partition)
        and accumulate sum(y^2) into a [1,512] psum row."""
        nch = len(ps_list)
        ev = []
        for c, ps in enumerate(ps_list):
            e = evp.tile([128, 512], MF, name="ev")
            em.copy(e[:], ps[:], None if bias_cols is None else bias_cols[c])
            ev.append(e)
        st = st_ps.tile([1, 512], F32, name="st", tag="stat_ps")
        for c in range(nch):
            sq = sqp.tile([128, 512], MF, name="sq")
            em.square(sq[:], ev[c][:], ev[c][:])
            nc.tensor.matmul(st[:], ones_col[:], sq[:],
                             start=(c == 0), stop=(c == nch - 1))
        return ev, st

    def ln_finish(ev, st, gam, bet, func, out_tile, dim=D):
        v = stp.tile([1, 512], F32, name="v", tag="ssb")
        nc.vector.tensor_scalar(v[0:1, :], st[:], 1.0 / dim, EPS,
                                mybir.AluOpType.mult, mybir.AluOpType.add)
        istd = emit_istd(v)
        bc = bcast(istd[:])
        for c, e in enumerate(ev):
            z = zp.tile([128, 512], MF, name="z")
            nc.vector.tensor_mul(z[:], e[:], bc[:])
            nc.scalar.activation(out_tile[:, c, :], z[:], func,
                                 bias=bet[:, c:c + 1], scale=gam[:, c:c + 1])

    def mm_group(n_m, srcs, bias_cols):
        """Emit an accumulating matmul group. srcs = list of (lhsT_fn, rhs)
        k-chunks; returns (ev, st) after evict+square+stats."""
        ps_list = [mm_ps.tile([128, 512], F32, name="mm") for _ in range(n_m)]
        last = len(srcs) - 1
        for ci, (lhsT_fn, rhs) in enumerate(srcs):
            for m in range(n_m):
                nc.tensor.matmul(ps_list[m][:], lhsT_fn(m), rhs,
                                 start=(ci == 0), stop=(ci == last))
        return ps_list

    # ---------------- main ----------------
    pend = []

    def flush(n=None):
        cnt = len(pend) if n is None else n
        for _ in range(cnt):
            if pend:
                pend.pop(0)()

    def wchunk(dram_ap):
        wc = wpool.tile([128, 512], MF, name="wc", tag="wc1")
        nc.sync.dma_start(wc[:], dram_ap)
        return wc

    def wpair(dram_pair_ap):
        """Load two [128,512] k-chunks in one DMA -> [128,2,512] tile."""
        wc = wpool.tile([128, 2, 512], MF, name="wcp", tag="wcp")
        nc.sync.dma_start(wc[:], dram_pair_ap.rearrange("c p n -> p c n"))
        return wc

    def pair_srcs(dram_4d, nk, rhs_fn):
        srcs = []
        for c0 in range(0, nk, 2):
            wc = wpair(dram_4d[c0:c0 + 2])
            for cc in range(2):
                srcs.append((lambda m, wc=wc, cc=cc: wc[:, cc, ts(m, 128)],
                             rhs_fn(c0 + cc)))
        return srcs

    for bt in range(NBT):
        bsl = ts(bt, 512)
        l_tiles = [None] * 3
        e_tiles = [None] * 3
        m_tiles = [None] * 3
        zt_tiles = [None] * 3
        w_rows = [None] * 3
        yh_tiles = [None] * 3

        # ---- lp matmuls (tiny) ----
        def emit_lp_mm(s):
            lt = ltp.tile([FS[s], 512], F32, name="lt", tag="lt")
            nc.sync.dma_start(lt[:], io[f"lT{s}"][:, bsl])
            lsg = ltp.tile([FS[s], 512], MF, name="lsg", tag="lsg")
            nc.scalar.activation(lsg[:], lt[:], ACT.Sigmoid)
            ps_list = [mm_ps.tile([128, 512], F32, name="mm") for _ in range(2)]
            for m in range(2):
                nc.tensor.matmul(ps_list[m][:], w_lp[s][:, ts(m, 128)],
                                 lsg[:], start=True, stop=True)
            ev, st = emit_ln(ps_list, [b_lp[:, s, c:c + 1] for c in range(2)])

            def fin(s=s, ev=ev, st=st):
                l_sb = lp_.tile([128, 2, 512], MF, name="l_sb")
                ln_finish(ev, st, g_lp[:, s], be_lp[:, s], ACT.Gelu, l_sb,
                          dim=D // 2)
                l_tiles[s] = l_sb
            pend.append(fin)

        def emit_hp_mm(s):
            xcs = []
            for c0 in range(0, 8, 2):
                xc = xpool.tile([128, 2, 512], MF, name="xc")
                nc.sync.dma_start(
                    xc[:], io[f"xT{s}"][ts(c0 // 2, 256), bsl].rearrange(
                        "(c p) b -> p c b", p=128))
                xcs.append(xc)
            srcs = pair_srcs(io["w_hp"][s], 8,
                             lambda c: xcs[c // 2][:, c % 2, :])
            ps_list = mm_group(4, srcs, None)
            ev, st = emit_ln(ps_list, [b_hp[:, s, c:c + 1] for c in range(4)])

            def fin(s=s, ev=ev, st=st):
                yh = yhp.tile([128, 4, 512], MF, name="yh")
                ln_finish(ev, st, g_hp[:, s], be_hp[:, s], ACT.Gelu, yh)
                yh_tiles[s] = yh
            pend.append(fin)

        def emit_r_mm(s):
            yh = yh_tiles[s]
            srcs = pair_srcs(io["w_r"], 4, lambda c: yh[:, c, :])
            ps_list = mm_group(4, srcs, None)
            ev, st = emit_ln(ps_list, [b_r[:, c:c + 1] for c in range(4)])

            def fin(s=s, ev=ev, st=st):
                e_sb = ep.tile([128, 4, 512], MF, name="e_sb")
                ln_finish(ev, st, g_n1[:, s], be_n1[:, s], ACT.Identity, e_sb)
                e_tiles[s] = e_sb
            pend.append(fin)

        m_streams = [(1, 2), (0, 2), (0, 1)]

        def emit_m_mm(s):
            sa, sb = m_streams[s]
            srcs = pair_srcs(io["w_m"][s], 8,
                             lambda ci: (e_tiles[sa][:, ci, :] if ci < 4
                                         else e_tiles[sb][:, ci - 4, :]))
            ps_list = mm_group(4, srcs, None)
            m_sb = mp.tile([128, 4, 512], MF, name="m_sb")
            for c in range(4):
                em.copy(m_sb[:, c, :], ps_list[c][:], b_m[:, s, c:c + 1])
            m_tiles[s] = m_sb

        def emit_gate_fuse(s):
            srcs = pair_srcs(io["w_g"][s], 8,
                             lambda ci: (e_tiles[s][:, ci, :] if ci < 4
                                         else m_tiles[s][:, ci - 4, :]))
            ps_list = mm_group(4, srcs, None)
            t_sb = tp.tile([128, 4, 512], MF, name="t_sb")
            for c in range(4):
                sg = sgp.tile([128, 512], MF, name="sg")
                nc.scalar.activation(sg[:], ps_list[c][:], ACT.Sigmoid,
                                     bias=b_g[:, s, c:c + 1])
                q = qp.tile([128, 512], MF, name="q")
                nc.vector.tensor_mul(q[:], sg[:], m_tiles[s][:, c, :])
                nc.vector.tensor_add(t_sb[:, c, :], e_tiles[s][:, c, :], q[:])
            st_sum = st_ps.tile([1, 512], F32, name="st_sum", tag="stat_ps")
            st_sq = st_ps.tile([1, 512], F32, name="st_sq", tag="stat_ps")
            for c in range(4):
                nc.tensor.matmul(st_sum[:], ones_col[:], t_sb[:, c, :],
                                 start=(c == 0), stop=(c == 3))
            for c in range(4):
                sq = sqp.tile([128, 512], MF, name="sq")
                em.square(sq[:], t_sb[:, c, :], t_sb[:, c, :])
                nc.tensor.matmul(st_sq[:], ones_col[:], sq[:],
                                 start=(c == 0), stop=(c == 3))

            def fin(s=s, t_sb=t_sb, st_sum=st_sum, st_sq=st_sq):
                mu = stp.tile([1, 512], F32, name="mu", tag="ssb")
                nc.vector.tensor_scalar_mul(mu[:], st_sum[:], 1.0 / D)
                ev2 = stp.tile([1, 512], F32, name="ev2", tag="ssb")
                nc.vector.tensor_scalar(ev2[:], st_sq[:], 1.0 / D, EPS,
                                        mybir.AluOpType.mult,
                                        mybir.AluOpType.add)
                v = stp.tile([1, 512], F32, name="v", tag="ssb")
                nc.vector.tensor_mul(v[:], mu[:], mu[:])
                nc.vector.tensor_sub(v[:], ev2[:], v[:])
                istd = emit_istd(v)
                w_row = stp.tile([1, 512], MF, name="w_row", tag="ssb")
                nc.vector.tensor_mul(w_row[:], mu[:], istd[:])
                w_rows[s] = w_row
                bc = bcast(istd[:])
                zt = ztp.tile([128, 4, 512], MF, name="zt")
                for c in range(4):
                    nc.vector.tensor_mul(zt[:, c, :], t_sb[:, c, :], bc[:])
                zt_tiles[s] = zt
            pend.append(fin)

        # ---------- emission schedule (software pipelined) ----------
        emit_lp_mm(0)
        emit_lp_mm(1)
        emit_lp_mm(2)
        emit_hp_mm(0)          # hp0 matmuls cover lp NR chains
        flush(2)               # lp0, lp1 fins
        emit_hp_mm(1)
        flush(2)               # lp2 fin + hp0 fin (covered by hp1 matmuls)
        emit_r_mm(0)
        flush(1)               # hp1 fin (covered by r'0/hp1 matmuls)
        emit_hp_mm(2)
        emit_r_mm(1)
        flush(1)               # n1_0 fin -> e0
        flush(1)               # hp2 fin -> yh2
        emit_r_mm(2)
        flush(1)               # n1_1 fin -> e1
        emit_m_mm(2)           # m_target needs e0,e1
        flush(1)               # n1_2 fin -> e2
        emit_gate_fuse(2)
        emit_m_mm(1)           # m_inst needs e0,e2
        flush(1)               # n2_2 fin -> zt2 (covered by m1 matmuls)
        emit_gate_fuse(1)
        emit_m_mm(0)           # m_verb needs e1,e2
        flush(1)               # n2_1 fin -> zt1 (covered by m0 matmuls)
        emit_gate_fuse(0)
        flush(1)               # n2_0 fin -> zt0 (covered by fus1 l/zt2/zt1)

        # ---- fus1: order k-chunks so zt0 (finished last) is consumed last
        srcs = []
        for s in range(3):
            srcs += pair_srcs(io["w_f1l"][s], 2,
                              lambda c, s=s: l_tiles[s][:, c, :])
        for s in (2, 1, 0):
            srcs += pair_srcs(io["w_f1"][s], 4,
                              lambda c, s=s: zt_tiles[s][:, c, :])
        for s in (2, 1, 0):
            srcs.append((lambda m, s=s: negc[0:1, s, ts(m, 128)],
                         w_rows[s][:]))
        ps_list = mm_group(4, srcs, None)
        ev, st = emit_ln(ps_list, [b_f1[:, c:c + 1] for c in range(4)])

        def fin_f1(ev=ev, st=st):
            h_sb = hp_.tile([128, 4, 512], MF, name="h_sb")
            ln_finish(ev, st, g_f1, be_f1, ACT.Gelu, h_sb)
            fin_f1.h = h_sb
        pend.append(fin_f1)
        flush(1)

        # ---- fus2
        h_sb = fin_f1.h
        srcs = pair_srcs(io["w_f2"], 4, lambda c: h_sb[:, c, :])
        ps_list = mm_group(4, srcs, None)
        ev, st = emit_ln(ps_list, [b_f2[:, c:c + 1] for c in range(4)])

        def fin_f2(ev=ev, st=st, bsl=bsl):
            o_sb = op_.tile([128, 4, 512], F32, name="o_sb")
            ln_finish(ev, st, g_f2, be_f2, ACT.Identity, o_sb)
            nc.sync.dma_start(
                io["outT"].rearrange("(c p) b -> p c b", p=128)[:, :, bsl],
                o_sb[:])
        pend.append(fin_f2)
        flush(1)

    flush()
    ctx.close()


def build_program():
    nc = bacc.Bacc("TRN2", target_bir_lowering=False, debug=False,
                   num_devices=NCORES)
    io = {}

    def din(name, shape, dtype=F32):
        io[name] = nc.dram_tensor(name, list(shape), dtype,
                                  kind="ExternalInput").ap()

    for s in range(3):
        din(f"xT{s}", (HID, BL), dtype=MM_DT)
        din(f"lT{s}", (FS[s], BL))
    din("w_hp", (3, 8, 128, 512), dtype=MM_DT)
    din("b_hp", (3, 128, 4))
    din("w_r", (4, 128, 512), dtype=MM_DT)
    din("b_r", (128, 4))
    din("w_m", (3, 8, 128, 512), dtype=MM_DT)
    din("b_m", (3, 128, 4))
    din("w_g", (3, 8, 128, 512), dtype=MM_DT)
    for s in range(3):
        din(f"w_lp{s}", (FS[s], 256), dtype=MM_DT)
    din("b_lp", (3, 128, 2))
    din("w_f1", (3, 4, 128, 512), dtype=MM_DT)
    din("w_f1l", (3, 2, 128, 512), dtype=MM_DT)
    din("negc_f1", (1, 3, 512), dtype=MM_DT)
    din("b_f1", (128, 4))
    din("w_f2", (4, 128, 512), dtype=MM_DT)
    din("b_f2", (128, 4))
    for name in ("g_hp", "be_hp", "g_n1", "be_n1", "b_g"):
        din(name, (3, 128, 4))
    for name in ("g_lp", "be_lp"):
        din(name, (3, 128, 2))
    for name in ("g_f1", "be_f1", "g_f2", "be_f2"):
        din(name, (128, 4))
    din("ones_col", (128, 1), dtype=MM_DT)
    io["outT"] = nc.dram_tensor("outT", [D, BL], F32,
                                kind="ExternalOutput").ap()
    if os.environ.get("KERNEL_DEBUG"):
        for s in range(3):
            for nm, shp in [(f"dbg_istd_hp{s}", [1, 512]),
                            (f"dbg_yh{s}", [128, 4, 512]),
                            (f"dbg_e{s}", [128, 4, 512]),
                            (f"dbg_l{s}", [128, 2, 512]),
                            (f"dbg_m{s}", [128, 4, 512]),
                            (f"dbg_t{s}", [128, 4, 512]),
                            (f"dbg_w{s}", [1, 512])]:
                io[nm] = nc.dram_tensor(nm, shp, F32,
                                        kind="ExternalOutput").ap()
        io["dbg_h"] = nc.dram_tensor("dbg_h", [128, 4, 512], F32,
                                     kind="ExternalOutput").ap()

    with tile.TileContext(nc) as tc:
        emit_program(tc, io)
    nc.compile()
    return nc


def make_in_maps(inputs):
    fw = fold_weights(inputs)
    dev = device_arrays(fw)
    hidden = [np.asarray(inputs["verb_hidden"], np.float32).T.astype(MM_NP),
              np.asarray(inputs["inst_hidden"], np.float32).T.astype(MM_NP),
              np.asarray(inputs["target_hidden"], np.float32).T.astype(MM_NP)]
    logits = [np.asarray(inputs["verb_logits"], np.float32),
              np.asarray(inputs["inst_logits"], np.float32),
              np.asarray(inputs["target_logits"], np.float32)]
    in_maps = []
    for core in range(NCORES):
        rows = slice(core * BL, (core + 1) * BL)
        m = dict(dev)
        for s in range(3):
            m[f"xT{s}"] = np.ascontiguousarray(hidden[s][:, rows])
            m[f"lT{s}"] = np.ascontiguousarray(logits[s][rows].T)
        in_maps.append(m)
    return in_maps


_NC_CACHE = None


def _run(inputs, **spmd_kwargs):
    global _NC_CACHE
    if _NC_CACHE is None:
        _NC_CACHE = build_program()
    nc = _NC_CACHE
    in_maps = make_in_maps(inputs)
    res = run_bass_kernel_spmd(nc, in_maps, list(range(NCORES)),
                               **spmd_kwargs)
    out = np.empty((B, D), dtype=np.float32)
    for core in range(NCORES):
        out[core * BL:(core + 1) * BL] = res.results[core]["outT"].T
    return out, res


def kernel(**inputs) -> np.ndarray:
    return _run(inputs)[0]


def kernel_profiled(inputs, tmpdir=None):
    """Returns (out, BassKernelResults) with an NTFF-based profile."""
    return _run(inputs, trace=True, tmpdir=tmpdir)



# revision 13
# speedup vs baseline: 1.5368x; 1.1429x over previous
"""Trainium2 Bass kernel for nn_AttentionModule_7146825580577.

Strategy (see spec sharding_hint): pure data parallel over the batch dim
(8192 rows -> 1024 rows per core, 8 cores), weights replicated.

Device math (per core), in feature-transposed layout (features on SBUF
partitions, batch on the free dim), fp16 matmul data with fp32 PSUM:

  - All LayerNorms whose input is an affine function of a previous
    activation use host-side column-centered weights, so mean(y) == 0 by
    construction and only sum(y^2) is needed on device (computed by a
    ones-vector matmul on the PE, reduced over partitions).
  - seq_len==1 MHA reduces to out_proj(v_proj(kv)); both projections are
    fused on the host into a single 512x512 effective matrix. The self-
    attention residual (x + sa(x)) is folded into a single matmul with
    weights I + Wv@Wo.
  - The cross-attention pair average (a+b)/2 is a single concat-matmul.
  - The n2 LayerNorm (after gating) is folded into the fus_W1 matmul:
    gamma scales fold into the weights, the per-sample mean correction is
    a rank-1 matmul term, betas fold into the bias.
  - 1/sqrt(var+eps) = ACT Sqrt of a single-instruction DVE approximate
    reciprocal (no PE transposes, ~18-bit accurate).
  - gelu is computed exactly via the Erf activation (gelu(x) =
    u*(1+erf(u*sqrt2)) with u = x/2 via pre-halved gamma/beta), so the
    scalar engine only ever needs two activation tables
    (sigmoid/erf/identity/square and sqrt) -- no table thrashing.
  - All matmul weights live in one packed fp16 DRAM tensor, DMAed once
    into a resident SBUF block; per-partition LN params live in one
    packed fp32 tensor. Total steady-state DMA: 3 input tensors per
    batch tile + 1 output.
  - Inputs are transposed on the host (numpy); the output is produced
    transposed in fp16 and transposed/upcast on the host.
"""
import os
import sys

sys.path.insert(0, "/opt/trn_rl_repo")

import numpy as np

import concourse.bass as bass
import concourse.tile as tile
from concourse import bacc, mybir
from concourse.bass import ts
from concourse.bass_utils import run_bass_kernel_spmd

D = 512
HID = 1024
B = 8192
NCORES = 8
BL = B // NCORES          # rows per core
NBT = BL // D             # batch tiles per core (2)
EPS = 1e-5
SQRT2 = 1.4142135623730951
F32 = mybir.dt.float32
MF = mybir.dt.float16     # matmul / vector-op dtype
MM_NP = np.float16
FS = [10, 6, 15]          # logit dims per stream

F64 = np.float64

# ---- packed weight block (fp16), offsets in elements (columns) ----
WLP = 0                      # 3 x 256 (partitions 0:FS[s])
WHP = WLP + 3 * 256          # 3 x 8 chunks x 512
WR = WHP + 3 * 8 * 512       # 4 x 512
WM = WR + 4 * 512            # 3 x 8 x 512
WG = WM + 3 * 8 * 512        # 3 x 8 x 512
WF1 = WG + 3 * 8 * 512       # 3 x 4 x 512
WF1L = WF1 + 3 * 4 * 512     # 3 x 2 x 512
WF2 = WF1L + 3 * 2 * 512     # 4 x 512
NEGC = WF2 + 4 * 512         # 3 x 512 (partition 0 only)
WTOT = NEGC + 3 * 512

# ---- packed per-partition params (fp32), column offsets ----
PB_HP, PB_R, PB_M, PB_LP, PB_F1, PB_F2 = 0, 12, 16, 28, 34, 38
PG_HP, PBE_HP, PG_N1, PBE_N1, PB_G = 42, 54, 66, 78, 90
PG_LP, PBE_LP, PG_F1, PBE_F1, PG_F2, PBE_F2 = 102, 108, 114, 118, 122, 126
PTOT = 130


# --------------------------------------------------------------------------
# Host-side weight folding
# --------------------------------------------------------------------------

def _center_cols(W, b):
    W = np.asarray(W, F64)
    b = np.asarray(b, F64)
    return W - W.mean(axis=1, keepdims=True), b - b.mean()


def fold_weights(inp):
    g = lambda k: np.asarray(inp[k], dtype=F64)
    out = {}

    w_hp, b_hp = [], []
    for s in range(3):
        W, b = _center_cols(g("hp_W")[s], g("hp_b")[s])
        w_hp.append(W)
        b_hp.append(b)
    out["w_hp"] = np.stack(w_hp)
    out["b_hp"] = np.stack(b_hp)
    out["g_hp"], out["be_hp"] = g("hp_g"), g("hp_be")

    mhaW, mhab = g("mha_in_W"), g("mha_in_b")
    moW, mob = g("mha_out_W"), g("mha_out_b")
    Wv0, bv0 = mhaW[0][:, 2 * D:], mhab[0][2 * D:]
    Wr, br = _center_cols(np.eye(D) + Wv0 @ moW[0], bv0 @ moW[0] + mob[0])
    out["w_r"], out["b_r"] = Wr, br
    out["g_n1"], out["be_n1"] = g("n1_g"), g("n1_be")

    Wj, bj = [None] * 4, [None] * 4
    for j in (1, 2, 3):
        Wv, bv = mhaW[j][:, 2 * D:], mhab[j][2 * D:]
        Wj[j] = Wv @ moW[j]
        bj[j] = bv @ moW[j] + mob[j]
    # m_verb uses (inst_e @ W1, target_e @ W2); m_inst (verb @ W1, target @ W3);
    # m_target (verb @ W2, inst @ W3)
    mods = [(1, 2), (1, 3), (2, 3)]
    w_m, b_m = [], []
    for s in range(3):
        ja, jb = mods[s]
        w_m.append(np.concatenate([0.5 * Wj[ja], 0.5 * Wj[jb]], axis=0))
        b_m.append(0.5 * (bj[ja] + bj[jb]))
    out["w_m"] = np.stack(w_m)
    out["b_m"] = np.stack(b_m)

    out["w_g"] = g("gate_W")
    out["b_g"] = g("gate_b")

    w_lp, b_lp = [], []
    for s, key in enumerate(["verb", "inst", "target"]):
        W, b = _center_cols(g(f"lp_W_{key}"), g(f"lp_b_{key}"))
        w_lp.append(W)
        b_lp.append(b)
    out["w_lp"] = w_lp
    out["b_lp"] = np.stack(b_lp)
    out["g_lp"], out["be_lp"] = g("lp_g"), g("lp_be")

    W1 = g("fus_W1")
    g2, be2 = g("n2_g"), g("n2_be")
    A1, negc = [], []
    bias_total = g("fus_b1").copy()
    for s in range(3):
        blk = W1[s * D:(s + 1) * D]
        A = g2[s][:, None] * blk
        c = blk.T @ g2[s]
        A1.append(A - A.mean(axis=1, keepdims=True))
        negc.append(-(c - c.mean()))
        bias_total += be2[s] @ blk
    L1 = []
    for s in range(3):
        off = 3 * D + s * (D // 2)
        blk = W1[off: off + D // 2]
        L1.append(blk - blk.mean(axis=1, keepdims=True))
    out["w_f1"] = np.stack(A1)
    out["negc_f1"] = np.stack(negc)
    out["w_f1l"] = np.stack(L1)
    out["b_f1"] = bias_total - bias_total.mean()
    out["g_f1"], out["be_f1"] = g("fus_g1"), g("fus_ge1")

    W2c, b2c = _center_cols(g("fus_W2"), g("fus_b2"))
    out["w_f2"], out["b_f2"] = W2c, b2c
    out["g_f2"], out["be_f2"] = g("fus_g2"), g("fus_ge2")
    return out


def _vec_pp(v, nk):
    """[.., nk*128] feature vector -> ACT per-partition layout [.., 128, nk]."""
    v = np.asarray(v, np.float32)
    return np.ascontiguousarray(v.reshape(v.shape[:-1] + (nk, 128)).swapaxes(-1, -2))


def pack_wall(fw):
    """All matmul weights -> one [128, WTOT] fp16 block.

    Each 512-col chunk c of a segment holds lhsT [128 K-partitions, 512]
    (4 M-tiles of 128 cols)."""
    wall = np.zeros((128, WTOT), MM_NP)

    def put(seg, w, nk):
        w = np.asarray(w, F64).reshape(nk, 128, 512)
        for c in range(nk):
            wall[:, seg + c * 512: seg + (c + 1) * 512] = w[c]

    for s in range(3):
        wall[:FS[s], WLP + s * 256: WLP + (s + 1) * 256] = \
            np.asarray(fw["w_lp"][s], F64)
    put(WHP, fw["w_hp"], 24)
    put(WR, fw["w_r"], 4)
    put(WM, fw["w_m"], 24)
    put(WG, fw["w_g"], 24)
    put(WF1, fw["w_f1"], 12)
    put(WF1L, fw["w_f1l"], 6)
    put(WF2, fw["w_f2"], 4)
    wall[0:1, NEGC: NEGC + 3 * 512] = \
        np.asarray(fw["negc_f1"], F64).reshape(1, 3 * 512)
    return wall


def pack_pars(fw):
    """All per-partition LN params -> one [128, PTOT] fp32 block.
    gamma/beta of gelu LayerNorms are pre-halved (erf-gelu identity)."""
    cols = []

    def p3(v, nk, half=False):
        a = _vec_pp(v, nk)            # [3,128,nk]
        a = a.transpose(1, 0, 2).reshape(128, 3 * nk)
        cols.append(a * 0.5 if half else a)

    def p2(v, nk, half=False):
        a = _vec_pp(v, nk)            # [128,nk]
        cols.append(a * 0.5 if half else a)

    p3(fw["b_hp"], 4)
    p2(fw["b_r"], 4)
    p3(fw["b_m"], 4)
    p3(fw["b_lp"], 2)
    p2(fw["b_f1"], 4)
    p2(fw["b_f2"], 4)
    p3(fw["g_hp"], 4, half=True)
    p3(fw["be_hp"], 4, half=True)
    p3(fw["g_n1"], 4)
    p3(fw["be_n1"], 4)
    p3(fw["b_g"], 4)
    p3(fw["g_lp"], 2, half=True)
    p3(fw["be_lp"], 2, half=True)
    p2(fw["g_f1"], 4, half=True)
    p2(fw["be_f1"], 4, half=True)
    p2(fw["g_f2"], 4)
    p2(fw["be_f2"], 4)
    pars = np.concatenate(cols, axis=1).astype(np.float32)
    assert pars.shape == (128, PTOT), pars.shape
    return np.ascontiguousarray(pars)


# --------------------------------------------------------------------------
# Device program
# --------------------------------------------------------------------------

class _Emit:
    def __init__(self, nc):
        self.nc = nc
        self.flip = 0

    def alt(self):
        """Alternate DVE / ACT for plain copies and squares."""
        self.flip ^= 1
        return self.flip

    def copy(self, out, in_, bias=None):
        """PSUM -> SBUF eviction, optionally adding a per-partition [128,1]
        bias column (the layer bias in transposed layout)."""
        nc = self.nc
        if self.alt():
            if bias is None:
                nc.vector.tensor_copy(out, in_)
            else:
                nc.vector.tensor_scalar_add(out, in_, bias)
        else:
            if bias is None:
                nc.scalar.activation(out, in_,
                                     mybir.ActivationFunctionType.Copy)
            else:
                nc.scalar.activation(out, in_,
                                     mybir.ActivationFunctionType.Identity,
                                     bias=bias)

    def square(self, out, in_sbuf, in_psum):
        """Square either from the evicted SBUF copy (DVE) or PSUM (ACT)."""
        nc = self.nc
        if self.alt():
            nc.vector.tensor_mul(out, in_sbuf, in_sbuf)
        else:
            nc.scalar.activation(out, in_psum,
                                 mybir.ActivationFunctionType.Square)


def emit_program(tc, io):
    nc = tc.nc
    from contextlib import ExitStack
    ctx = ExitStack()
    em = _Emit(nc)
    ACT = mybir.ActivationFunctionType
    ALU = mybir.AluOpType

    # ---------------- pools ----------------
    P = lambda name, bufs, space="SBUF": ctx.enter_context(
        tc.tile_pool(name=name, bufs=bufs, space=space))
    const = P("const", 1)
    xpool = P("xchunk", 2)
    evp = P("ev", 9)
    sqp = P("sq", 2)
    zp = P("z", 2)
    up = P("u", 2)
    erp = P("er", 2)
    yhp = P("yh", 1)
    ep = P("e", 3)
    mp = P("m", 2)
    sgp = P("sg", 1)
    qp = P("q", 1)
    tp = P("t", 3)
    lp_ = P("l", 3)
    hp_ = P("h", 1)
    op_ = P("o", 1)
    stp = P("stats_sb", 6)
    bcp = P("bc_sb", 2)
    ltp = P("lt", 1)
    lsgp = P("lsg", 2)
    mm_ps = P("mm_ps", 6, "PSUM")
    st_ps = P("st_ps", 2, "PSUM")

    # ---------------- resident weights / params ----------------
    wall = const.tile([128, WTOT], MF, name="wall")
    pars = const.tile([128, PTOT], F32, name="pars")
    ones_col = const.tile([128, 1], MF, name="ones_col")
    nc.gpsimd.memset(ones_col[:], 1.0)

    def preload_weights():
        sd = nc.scalar.dma_start
        sd(pars[:], io["pars"])
        sd(wall[:, :WHP], io["wall"][:, :WHP])          # w_lp (tiny)
        for s in range(3):                              # w_hp per stream
            a, b = WHP + s * 8 * 512, WHP + (s + 1) * 8 * 512
            sd(wall[:, a:b], io["wall"][:, a:b])
        sd(wall[:, WR:], io["wall"][:, WR:])            # all the rest

    def wch(seg, c, m):
        base = seg + c * 512 + m * 128
        return wall[:, base: base + 128]

    def wsrcs(seg, nk, rhs_fn):
        return [(lambda m, c=c: wch(seg, c, m), rhs_fn(c))
                for c in range(nk)]

    def pcol(k):
        return pars[:, k:k + 1]

    def pblk(k, n):
        return pars[:, k:k + n]

    # ---------------- helpers ----------------
    def emit_istd(v_sb):
        """v_sb: [1,512] sbuf fp32 variance (+eps already added).
        Returns a [1,512] MF istd row via DVE approx-reciprocal + ACT sqrt
        (rsqrt(v) = sqrt(1/v)); ~18-bit accurate, no PE transposes."""
        r = stp.tile([1, 512], F32, name="recip", tag="ssb")
        nc.vector.reciprocal_approx_fast(out=r[:], in_=v_sb[0:1, :])
        istd = stp.tile([1, 512], MF, name="istd", tag="ssb")
        nc.scalar.activation(istd[:], r[:], ACT.Sqrt)
        return istd

    def bcast(row_ap):
        """[1,512] sbuf row -> [128,512] sbuf tile via GPSIMD."""
        bc = bcp.tile([128, 512], MF, name="bc")
        nc.gpsimd.partition_broadcast(bc[:], row_ap)
        return bc

    def emit_ln(ps_list, bias_cols=None):
        """Evict psum chunks to SBUF (adding the layer bias per partition)
        and accumulate sum(y^2) into a [1,512] psum row."""
        nch = len(ps_list)
        ev = []
        for c, ps in enumerate(ps_list):
            e = evp.tile([128, 512], MF, name="ev")
            em.copy(e[:], ps[:], None if bias_cols is None else bias_cols[c])
            ev.append(e)
        st = st_ps.tile([1, 512], F32, name="st", tag="stat_ps")
        for c in range(nch):
            sq = sqp.tile([128, 512], MF, name="sq")
            em.square(sq[:], ev[c][:], ev[c][:])
            nc.tensor.matmul(st[:], ones_col[:], sq[:],
                             start=(c == 0), stop=(c == nch - 1))
        return ev, st

    def ln_finish(ev, st, gam, bet, gelu, out_tile, dim=D):
        """gam/bet: [128,nch] AP (pre-halved if gelu). gelu computes the
        exact erf form: out = u*(1+erf(u*sqrt2)), u = (z*g+b)/2."""
        v = stp.tile([1, 512], F32, name="v", tag="ssb")
        nc.vector.tensor_scalar(v[0:1, :], st[:], 1.0 / dim, EPS,
                                ALU.mult, ALU.add)
        istd = emit_istd(v)
        bc = bcast(istd[:])
        for c, e in enumerate(ev):
            z = zp.tile([128, 512], MF, name="z")
            nc.vector.tensor_mul(z[:], e[:], bc[:])
            if gelu:
                u = up.tile([128, 512], MF, name="u")
                nc.scalar.activation(u[:], z[:], ACT.Identity,
                                     bias=bet[:, c:c + 1],
                                     scale=gam[:, c:c + 1])
                er = erp.tile([128, 512], MF, name="er")
                nc.scalar.activation(er[:], u[:], ACT.Erf, scale=SQRT2)
                nc.vector.scalar_tensor_tensor(out_tile[:, c, :], er[:], 1.0,
                                               u[:], ALU.add, ALU.mult)
            else:
                nc.scalar.activation(out_tile[:, c, :], z[:], ACT.Identity,
                                     bias=bet[:, c:c + 1],
                                     scale=gam[:, c:c + 1])

    def mm_group(n_m, srcs):
        """Emit an accumulating matmul group. srcs = list of (lhsT_fn, rhs)
        k-chunks; returns the psum tiles."""
        ps_list = [mm_ps.tile([128, 512], F32, name="mm") for _ in range(n_m)]
        last = len(srcs) - 1
        for ci, (lhsT_fn, rhs) in enumerate(srcs):
            for m in range(n_m):
                nc.tensor.matmul(ps_list[m][:], lhsT_fn(m), rhs,
                                 start=(ci == 0), stop=(ci == last))
        return ps_list

    # ---------------- per-bt state ----------------
    S = [dict(lt=[None] * 3, xc=[None] * 3, l=[None] * 3, yh=[None] * 3,
              e=[None] * 3, m=[None] * 3, zt=[None] * 3, wr=[None] * 3,
              h=None) for _ in range(NBT)]
    pend = []

    def flush(n=None):
        cnt = len(pend) if n is None else n
        for _ in range(cnt):
            if pend:
                pend.pop(0)()

    def emit_x_loads(bt):
        bsl = ts(bt, 512)
        for s in range(3):
            lt = ltp.tile([FS[s], 512], F32, name="lt")
            nc.sync.dma_start(lt[:], io[f"lT{s}"][:, bsl])
            S[bt]["lt"][s] = lt
        for s in range(3):
            xc = xpool.tile([128, 8, 512], MF, name="xc")
            nc.sync.dma_start(
                xc[:], io[f"xT{s}"][:, bsl].rearrange("(c p) b -> p c b",
                                                      p=128))
            S[bt]["xc"][s] = xc

    # ---------------- phase emitters ----------------
    def emit_lp_mm(bt, s):
        lsg = lsgp.tile([FS[s], 512], MF, name="lsg")
        nc.scalar.activation(lsg[:], S[bt]["lt"][s][:], ACT.Sigmoid)
        ps_list = [mm_ps.tile([128, 512], F32, name="mm") for _ in range(2)]
        for m in range(2):
            nc.tensor.matmul(ps_list[m][:],
                             wall[0:FS[s], WLP + s * 256 + m * 128:
                                  WLP + s * 256 + (m + 1) * 128],
                             lsg[:], start=True, stop=True)
        ev, st = emit_ln(ps_list, [pcol(PB_LP + s * 2 + c) for c in range(2)])

        def fin(s=s, bt=bt, ev=ev, st=st):
            l_sb = lp_.tile([128, 2, 512], MF, name="l_sb")
            ln_finish(ev, st, pblk(PG_LP + s * 2, 2), pblk(PBE_LP + s * 2, 2),
                      True, l_sb, dim=D // 2)
            S[bt]["l"][s] = l_sb
        pend.append(fin)

    def emit_hp_mm(bt, s):
        xc = S[bt]["xc"][s]
        srcs = [(lambda m, c=c: wch(WHP, s * 8 + c, m), xc[:, c, :])
                for c in range(8)]
        ps_list = mm_group(4, srcs)
        ev, st = emit_ln(ps_list, [pcol(PB_HP + s * 4 + c) for c in range(4)])

        def fin(s=s, bt=bt, ev=ev, st=st):
            yh = yhp.tile([128, 4, 512], MF, name="yh")
            ln_finish(ev, st, pblk(PG_HP + s * 4, 4), pblk(PBE_HP + s * 4, 4),
                      True, yh)
            S[bt]["yh"][s] = yh
        pend.append(fin)

    def emit_r_mm(bt, s):
        yh = S[bt]["yh"][s]
        srcs = wsrcs(WR, 4, lambda c: yh[:, c, :])
        ps_list = mm_group(4, srcs)
        ev, st = emit_ln(ps_list, [pcol(PB_R + c) for c in range(4)])

        def fin(s=s, bt=bt, ev=ev, st=st):
            e_sb = ep.tile([128, 4, 512], MF, name="e_sb")
            ln_finish(ev, st, pblk(PG_N1 + s * 4, 4), pblk(PBE_N1 + s * 4, 4),
                      False, e_sb)
            S[bt]["e"][s] = e_sb
        pend.append(fin)

    m_streams = [(1, 2), (0, 2), (0, 1)]

    def emit_m_mm(bt, s):
        sa, sb = m_streams[s]
        e_tiles = S[bt]["e"]
        srcs = [(lambda m, c=c: wch(WM, s * 8 + c, m),
                 (e_tiles[sa][:, c, :] if c < 4 else e_tiles[sb][:, c - 4, :]))
                for c in range(8)]
        ps_list = mm_group(4, srcs)
        m_sb = mp.tile([128, 4, 512], MF, name="m_sb")
        for c in range(4):
            em.copy(m_sb[:, c, :], ps_list[c][:], pcol(PB_M + s * 4 + c))
        S[bt]["m"][s] = m_sb

    def emit_gate_fuse(bt, s):
        e_sb, m_sb = S[bt]["e"][s], S[bt]["m"][s]
        srcs = [(lambda m, c=c: wch(WG, s * 8 + c, m),
                 (e_sb[:, c, :] if c < 4 else m_sb[:, c - 4, :]))
                for c in range(8)]
        ps_list = mm_group(4, srcs)
        t_sb = tp.tile([128, 4, 512], MF, name="t_sb")
        for c in range(4):
            sg = sgp.tile([128, 512], MF, name="sg")
            nc.scalar.activation(sg[:], ps_list[c][:], ACT.Sigmoid,
                                 bias=pcol(PB_G + s * 4 + c))
            q = qp.tile([128, 512], MF, name="q")
            nc.vector.tensor_mul(q[:], sg[:], m_sb[:, c, :])
            nc.vector.tensor_add(t_sb[:, c, :], e_sb[:, c, :], q[:])
        st_sum = st_ps.tile([1, 512], F32, name="st_sum", tag="stat_ps")
        st_sq = st_ps.tile([1, 512], F32, name="st_sq", tag="stat_ps")
        for c in range(4):
            nc.tensor.matmul(st_sum[:], ones_col[:], t_sb[:, c, :],
                             start=(c == 0), stop=(c == 3))
        for c in range(4):
            sq = sqp.tile([128, 512], MF, name="sq")
            em.square(sq[:], t_sb[:, c, :], t_sb[:, c, :])
            nc.tensor.matmul(st_sq[:], ones_col[:], sq[:],
                             start=(c == 0), stop=(c == 3))

        def fin(s=s, bt=bt, t_sb=t_sb, st_sum=st_sum, st_sq=st_sq):
            mu = stp.tile([1, 512], F32, name="mu", tag="ssb")
            nc.vector.tensor_scalar_mul(mu[:], st_sum[:], 1.0 / D)
            ev2 = stp.tile([1, 512], F32, name="ev2", tag="ssb")
            nc.vector.tensor_scalar(ev2[:], st_sq[:], 1.0 / D, EPS,
                                    ALU.mult, ALU.add)
            v = stp.tile([1, 512], F32, name="v", tag="ssb")
            nc.vector.tensor_mul(v[:], mu[:], mu[:])
            nc.vector.tensor_sub(v[:], ev2[:], v[:])
            istd = emit_istd(v)
            w_row = stp.tile([1, 512], MF, name="w_row", tag="ssb")
            nc.vector.tensor_mul(w_row[:], mu[:], istd[:])
            S[bt]["wr"][s] = w_row
            bc = bcast(istd[:])
            for c in range(4):
                nc.vector.tensor_mul(t_sb[:, c, :], t_sb[:, c, :], bc[:])
            S[bt]["zt"][s] = t_sb
        pend.append(fin)

    def emit_f1(bt):
        # order k-chunks so zt0 (finished last) is consumed last
        srcs = []
        for s in range(3):
            srcs += [(lambda m, c=c, s=s: wch(WF1L, s * 2 + c, m),
                      S[bt]["l"][s][:, c, :]) for c in range(2)]
        for s in (2, 1, 0):
            srcs += [(lambda m, c=c, s=s: wch(WF1, s * 4 + c, m),
                      S[bt]["zt"][s][:, c, :]) for c in range(4)]
        for s in (2, 1, 0):
            srcs.append((lambda m, s=s: wall[0:1, NEGC + s * 512 + m * 128:
                                             NEGC + s * 512 + (m + 1) * 128],
                         S[bt]["wr"][s][:]))
        ps_list = mm_group(4, srcs)
        ev, st = emit_ln(ps_list, [pcol(PB_F1 + c) for c in range(4)])

        def fin(bt=bt, ev=ev, st=st):
            h_sb = hp_.tile([128, 4, 512], MF, name="h_sb")
            ln_finish(ev, st, pblk(PG_F1, 4), pblk(PBE_F1, 4), True, h_sb)
            S[bt]["h"] = h_sb
        pend.append(fin)

    def emit_f2(bt):
        h_sb = S[bt]["h"]
        srcs = wsrcs(WF2, 4, lambda c: h_sb[:, c, :])
        ps_list = mm_group(4, srcs)
        ev, st = emit_ln(ps_list, [pcol(PB_F2 + c) for c in range(4)])

        def fin(bt=bt, ev=ev, st=st):
            bsl = ts(bt, 512)
            o_sb = op_.tile([128, 4, 512], MF, name="o_sb")
            ln_finish(ev, st, pblk(PG_F2, 4), pblk(PBE_F2, 4), False, o_sb)
            nc.sync.dma_start(
                io["outT"].rearrange("(c p) b -> p c b", p=128)[:, :, bsl],
                o_sb[:])
        pend.append(fin)

    # ---------------- schedule (software pipelined across both bts) -----
    emit_x_loads(0)
    preload_weights()

    def mid_phases(bt):
        emit_r_mm(bt, 0)
        flush(1)               # hp1 fin
        emit_hp_mm(bt, 2)
        emit_r_mm(bt, 1)
        flush(1)               # n1_0 fin -> e0
        flush(1)               # hp2 fin -> yh2
        emit_r_mm(bt, 2)
        flush(1)               # n1_1 fin -> e1
        emit_m_mm(bt, 2)       # m_target needs e0,e1
        flush(1)               # n1_2 fin -> e2
        emit_gate_fuse(bt, 2)
        emit_m_mm(bt, 1)       # m_inst needs e0,e2
        flush(1)               # n2_2 fin -> zt2 (covered by m1 matmuls)
        emit_gate_fuse(bt, 1)
        emit_m_mm(bt, 0)       # m_verb needs e1,e2
        flush(1)               # n2_1 fin -> zt1 (covered by m0 matmuls)
        emit_gate_fuse(bt, 0)
        flush(1)               # n2_0 fin -> zt0 (covered by f1 l/zt2/zt1)

    # ---- bt 0 ----
    for s in range(3):
        emit_lp_mm(0, s)
    emit_hp_mm(0, 0)           # hp0 matmuls cover lp fin chains
    flush(2)                   # lp0, lp1 fins
    emit_hp_mm(0, 1)
    flush(2)                   # lp2 fin + hp0 fin
    mid_phases(0)
    emit_x_loads(1)            # prefetch bt1 inputs
    emit_f1(0)
    # ---- bt1 head overlaps bt0 f1/f2 LN tails ----
    for s in range(3):
        emit_lp_mm(1, s)
    emit_hp_mm(1, 0)
    flush(1)                   # f1(0) fin -> h (covered by bt1 lp/hp mms)
    emit_f2(0)
    emit_hp_mm(1, 1)
    flush(2)                   # lp0(1), lp1(1) fins
    flush(2)                   # lp2(1), hp0(1) fins
    emit_r_mm(1, 0)
    flush(1)                   # f2(0) fin + output DMA (covered by r/hp mms)
    # ---- bt 1 remainder (mirrors bt0 mid/tail) ----
    flush(1)                   # hp1(1) fin
    emit_hp_mm(1, 2)
    emit_r_mm(1, 1)
    flush(1)                   # e0
    flush(1)                   # yh2
    emit_r_mm(1, 2)
    flush(1)                   # e1
    emit_m_mm(1, 2)
    flush(1)                   # e2
    emit_gate_fuse(1, 2)
    emit_m_mm(1, 1)
    flush(1)                   # zt2
    emit_gate_fuse(1, 1)
    emit_m_mm(1, 0)
    flush(1)                   # zt1
    emit_gate_fuse(1, 0)
    flush(1)                   # zt0
    emit_f1(1)
    flush(1)                   # f1(1) fin
    emit_f2(1)
    flush()
    ctx.close()


def build_program():
    nc = bacc.Bacc("TRN2", target_bir_lowering=False, debug=False,
                   num_devices=NCORES)
    io = {}

    def din(name, shape, dtype=F32):
        io[name] = nc.dram_tensor(name, list(shape), dtype,
                                  kind="ExternalInput").ap()

    for s in range(3):
        din(f"xT{s}", (HID, BL), dtype=MF)
        din(f"lT{s}", (FS[s], BL))
    din("wall", (128, WTOT), dtype=MF)
    din("pars", (128, PTOT))
    io["outT"] = nc.dram_tensor("outT", [D, BL], MF,
                                kind="ExternalOutput").ap()

    with tile.TileContext(nc) as tc:
        emit_program(tc, io)
    nc.compile()
    return nc


def make_in_maps(inputs):
    fw = fold_weights(inputs)
    wall = pack_wall(fw)
    pars = pack_pars(fw)
    hidden = [np.asarray(inputs["verb_hidden"], np.float32).T.astype(MM_NP),
              np.asarray(inputs["inst_hidden"], np.float32).T.astype(MM_NP),
              np.asarray(inputs["target_hidden"], np.float32).T.astype(MM_NP)]
    logits = [np.asarray(inputs["verb_logits"], np.float32),
              np.asarray(inputs["inst_logits"], np.float32),
              np.asarray(inputs["target_logits"], np.float32)]
    in_maps = []
    for core in range(NCORES):
        rows = slice(core * BL, (core + 1) * BL)
        m = {"wall": wall, "pars": pars}
        for s in range(3):
            m[f"xT{s}"] = np.ascontiguousarray(hidden[s][:, rows])
            m[f"lT{s}"] = np.ascontiguousarray(logits[s][rows].T)
        in_maps.append(m)
    return in_maps


_NC_CACHE = None


def _run(inputs, **spmd_kwargs):
    global _NC_CACHE
    if _NC_CACHE is None:
        _NC_CACHE = build_program()
    nc = _NC_CACHE
    in_maps = make_in_maps(inputs)
    res = run_bass_kernel_spmd(nc, in_maps, list(range(NCORES)),
                               **spmd_kwargs)
    out = np.empty((B, D), dtype=np.float32)
    for core in range(NCORES):
        out[core * BL:(core + 1) * BL] = \
            res.results[core]["outT"].T.astype(np.float32)
    return out, res


def kernel(**inputs) -> np.ndarray:
    return _run(inputs)[0]


def kernel_profiled(inputs, tmpdir=None):
    """Returns (out, BassKernelResults) with an NTFF-based profile."""
    return _run(inputs, trace=True, tmpdir=tmpdir)


# revision 14
# speedup vs baseline: 1.7737x; 1.1542x over previous
"""Trainium2 Bass kernel for nn_AttentionModule_7146825580577.

Strategy (see spec sharding_hint): pure data parallel over the batch dim
(8192 rows -> 1024 rows per core, 8 cores), weights replicated.

Device math (per core), in feature-transposed layout (features on SBUF
partitions, batch on the free dim), fp16 matmul data with fp32 PSUM:

  - All LayerNorms whose input is an affine function of a previous
    activation use host-side column-centered weights, so mean(y) == 0 by
    construction and only sum(y^2) is needed on device (computed by a
    ones-vector matmul on the PE, reduced over partitions).
  - seq_len==1 MHA reduces to out_proj(v_proj(kv)); both projections are
    fused on the host into a single 512x512 effective matrix. The self-
    attention residual (x + sa(x)) is folded into a single matmul with
    weights I + Wv@Wo.
  - The cross-attention pair average (a+b)/2 is a single concat-matmul.
  - The n2 LayerNorm (after gating) is folded into the fus_W1 matmul:
    gamma scales fold into the weights, the per-sample mean correction is
    a rank-1 matmul term, betas fold into the bias.
  - 1/sqrt(var+eps) = ACT Sqrt of a single-instruction DVE approximate
    reciprocal (no PE transposes, ~18-bit accurate).
  - gelu is computed exactly via the Erf activation (gelu(x) =
    u*(1+erf(u*sqrt2)) with u = x/2 via pre-halved gamma/beta), so the
    scalar engine only ever needs two activation tables
    (sigmoid/erf/identity/square and sqrt) -- no table thrashing.
  - All matmul weights live in one packed fp16 DRAM tensor, DMAed once
    into a resident SBUF block; per-partition LN params live in one
    packed fp32 tensor. Total steady-state DMA: 3 input tensors per
    batch tile + 1 output.
  - Inputs are transposed on the host (numpy); the output is produced
    transposed in fp16 and transposed/upcast on the host.
"""
import os
import sys

sys.path.insert(0, "/opt/trn_rl_repo")

import numpy as np

import concourse.bass as bass
import concourse.tile as tile
from concourse import bacc, mybir
from concourse.bass import ts
from concourse.bass_utils import run_bass_kernel_spmd

D = 512
HID = 1024
B = 8192
NCORES = 8
BL = B // NCORES          # rows per core
NBT = BL // D             # batch tiles per core (2)
EPS = 1e-5
SQRT2 = 1.4142135623730951
F32 = mybir.dt.float32
MF = mybir.dt.float16     # matmul / vector-op dtype
MM_NP = np.float16
FS = [10, 6, 15]          # logit dims per stream

F64 = np.float64

# ---- packed weight block (fp16), offsets in elements (columns) ----
WLP = 0                      # 3 x 256 (partitions 0:FS[s])
WHP = WLP + 3 * 256          # 3 x 8 chunks x 512
WR = WHP + 3 * 8 * 512       # 4 x 512
WM = WR + 4 * 512            # 3 x 8 x 512
WG = WM + 3 * 8 * 512        # 3 x 8 x 512
WF1 = WG + 3 * 8 * 512       # 3 x 4 x 512
WF1L = WF1 + 3 * 4 * 512     # 3 x 2 x 512
WF2 = WF1L + 3 * 2 * 512     # 4 x 512
NEGC = WF2 + 4 * 512         # 3 x 512 (partition 0 only)
WTOT = NEGC + 3 * 512

# ---- packed per-partition params (fp32), column offsets ----
PB_HP, PB_R, PB_M, PB_LP, PB_F1, PB_F2 = 0, 12, 16, 28, 34, 38
PG_HP, PBE_HP, PG_N1, PBE_N1, PB_G = 42, 54, 66, 78, 90
PG_LP, PBE_LP, PG_F1, PBE_F1, PG_F2, PBE_F2 = 102, 108, 114, 118, 122, 126
PTOT = 130


# --------------------------------------------------------------------------
# Host-side weight folding
# --------------------------------------------------------------------------

def _center_cols(W, b):
    W = np.asarray(W, F64)
    b = np.asarray(b, F64)
    return W - W.mean(axis=1, keepdims=True), b - b.mean()


def fold_weights(inp):
    g = lambda k: np.asarray(inp[k], dtype=F64)
    out = {}

    w_hp, b_hp = [], []
    for s in range(3):
        W, b = _center_cols(g("hp_W")[s], g("hp_b")[s])
        w_hp.append(W)
        b_hp.append(b)
    out["w_hp"] = np.stack(w_hp)
    out["b_hp"] = np.stack(b_hp)
    out["g_hp"], out["be_hp"] = g("hp_g"), g("hp_be")

    mhaW, mhab = g("mha_in_W"), g("mha_in_b")
    moW, mob = g("mha_out_W"), g("mha_out_b")
    Wv0, bv0 = mhaW[0][:, 2 * D:], mhab[0][2 * D:]
    Wr, br = _center_cols(np.eye(D) + Wv0 @ moW[0], bv0 @ moW[0] + mob[0])
    out["w_r"], out["b_r"] = Wr, br
    out["g_n1"], out["be_n1"] = g("n1_g"), g("n1_be")

    Wj, bj = [None] * 4, [None] * 4
    for j in (1, 2, 3):
        Wv, bv = mhaW[j][:, 2 * D:], mhab[j][2 * D:]
        Wj[j] = Wv @ moW[j]
        bj[j] = bv @ moW[j] + mob[j]
    # m_verb uses (inst_e @ W1, target_e @ W2); m_inst (verb @ W1, target @ W3);
    # m_target (verb @ W2, inst @ W3)
    mods = [(1, 2), (1, 3), (2, 3)]
    w_m, b_m = [], []
    for s in range(3):
        ja, jb = mods[s]
        w_m.append(np.concatenate([0.5 * Wj[ja], 0.5 * Wj[jb]], axis=0))
        b_m.append(0.5 * (bj[ja] + bj[jb]))
    out["w_m"] = np.stack(w_m)
    out["b_m"] = np.stack(b_m)

    out["w_g"] = g("gate_W")
    out["b_g"] = g("gate_b")

    w_lp, b_lp = [], []
    for s, key in enumerate(["verb", "inst", "target"]):
        W, b = _center_cols(g(f"lp_W_{key}"), g(f"lp_b_{key}"))
        w_lp.append(W)
        b_lp.append(b)
    out["w_lp"] = w_lp
    out["b_lp"] = np.stack(b_lp)
    out["g_lp"], out["be_lp"] = g("lp_g"), g("lp_be")

    W1 = g("fus_W1")
    g2, be2 = g("n2_g"), g("n2_be")
    A1, negc = [], []
    bias_total = g("fus_b1").copy()
    for s in range(3):
        blk = W1[s * D:(s + 1) * D]
        A = g2[s][:, None] * blk
        c = blk.T @ g2[s]
        A1.append(A - A.mean(axis=1, keepdims=True))
        negc.append(-(c - c.mean()))
        bias_total += be2[s] @ blk
    L1 = []
    for s in range(3):
        off = 3 * D + s * (D // 2)
        blk = W1[off: off + D // 2]
        L1.append(blk - blk.mean(axis=1, keepdims=True))
    out["w_f1"] = np.stack(A1)
    out["negc_f1"] = np.stack(negc)
    out["w_f1l"] = np.stack(L1)
    out["b_f1"] = bias_total - bias_total.mean()
    out["g_f1"], out["be_f1"] = g("fus_g1"), g("fus_ge1")

    W2c, b2c = _center_cols(g("fus_W2"), g("fus_b2"))
    out["w_f2"], out["b_f2"] = W2c, b2c
    out["g_f2"], out["be_f2"] = g("fus_g2"), g("fus_ge2")
    return out


def _vec_pp(v, nk):
    """[.., nk*128] feature vector -> ACT per-partition layout [.., 128, nk]."""
    v = np.asarray(v, np.float32)
    return np.ascontiguousarray(v.reshape(v.shape[:-1] + (nk, 128)).swapaxes(-1, -2))


def pack_wall(fw):
    """All matmul weights -> one [128, WTOT] fp16 block.

    Each 512-col chunk c of a segment holds lhsT [128 K-partitions, 512]
    (4 M-tiles of 128 cols)."""
    wall = np.zeros((128, WTOT), MM_NP)

    def put(seg, w, nk):
        w = np.asarray(w, F64).reshape(nk, 128, 512)
        for c in range(nk):
            wall[:, seg + c * 512: seg + (c + 1) * 512] = w[c]

    for s in range(3):
        wall[:FS[s], WLP + s * 256: WLP + (s + 1) * 256] = \
            np.asarray(fw["w_lp"][s], F64)
    put(WHP, fw["w_hp"], 24)
    put(WR, fw["w_r"], 4)
    put(WM, fw["w_m"], 24)
    put(WG, fw["w_g"], 24)
    put(WF1, fw["w_f1"], 12)
    put(WF1L, fw["w_f1l"], 6)
    put(WF2, fw["w_f2"], 4)
    wall[0:1, NEGC: NEGC + 3 * 512] = \
        np.asarray(fw["negc_f1"], F64).reshape(1, 3 * 512)
    return wall


def pack_pars(fw):
    """All per-partition LN params -> one [128, PTOT] fp32 block."""
    cols = []

    def p3(v, nk, half=False):
        a = _vec_pp(v, nk)            # [3,128,nk]
        a = a.transpose(1, 0, 2).reshape(128, 3 * nk)
        cols.append(a * 0.5 if half else a)

    def p2(v, nk, half=False):
        a = _vec_pp(v, nk)            # [128,nk]
        cols.append(a * 0.5 if half else a)

    p3(fw["b_hp"], 4)
    p2(fw["b_r"], 4)
    p3(fw["b_m"], 4)
    p3(fw["b_lp"], 2)
    p2(fw["b_f1"], 4)
    p2(fw["b_f2"], 4)
    p3(fw["g_hp"], 4)
    p3(fw["be_hp"], 4)
    p3(fw["g_n1"], 4)
    p3(fw["be_n1"], 4)
    p3(fw["b_g"], 4)
    p3(fw["g_lp"], 2)
    p3(fw["be_lp"], 2)
    p2(fw["g_f1"], 4)
    p2(fw["be_f1"], 4)
    p2(fw["g_f2"], 4)
    p2(fw["be_f2"], 4)
    pars = np.concatenate(cols, axis=1).astype(np.float32)
    assert pars.shape == (128, PTOT), pars.shape
    return np.ascontiguousarray(pars)


# --------------------------------------------------------------------------
# Device program
# --------------------------------------------------------------------------

class _Emit:
    def __init__(self, nc):
        self.nc = nc
        self.flip = 0

    def alt(self):
        """Alternate DVE / ACT for plain copies and squares."""
        self.flip ^= 1
        return self.flip

    def copy(self, out, in_, bias=None):
        """PSUM -> SBUF eviction, optionally adding a per-partition [128,1]
        bias column (the layer bias in transposed layout)."""
        nc = self.nc
        if self.alt():
            if bias is None:
                nc.vector.tensor_copy(out, in_)
            else:
                nc.vector.tensor_scalar_add(out, in_, bias)
        else:
            if bias is None:
                nc.scalar.activation(out, in_,
                                     mybir.ActivationFunctionType.Copy)
            else:
                nc.scalar.activation(out, in_,
                                     mybir.ActivationFunctionType.Identity,
                                     bias=bias)

    def square(self, out, in_sbuf, in_psum):
        """Square either from the evicted SBUF copy (DVE) or PSUM (ACT)."""
        nc = self.nc
        if self.alt():
            nc.vector.tensor_mul(out, in_sbuf, in_sbuf)
        else:
            nc.scalar.activation(out, in_psum,
                                 mybir.ActivationFunctionType.Square)


def emit_program(tc, io):
    nc = tc.nc
    from contextlib import ExitStack
    ctx = ExitStack()
    em = _Emit(nc)
    ACT = mybir.ActivationFunctionType
    ALU = mybir.AluOpType

    # ---------------- pools ----------------
    P = lambda name, bufs, space="SBUF": ctx.enter_context(
        tc.tile_pool(name=name, bufs=bufs, space=space))
    const = P("const", 1)
    xpool = P("xchunk", 2)
    evp = P("ev", 9)
    sqp = P("sq", 2)
    zp = P("z", 2)
    yhp = P("yh", 1)
    ep = P("e", 3)
    mp = P("m", 2)
    sgp = P("sg", 1)
    qp = P("q", 1)
    tp = P("t", 3)
    lp_ = P("l", 3)
    hp_ = P("h", 1)
    op_ = P("o", 1)
    stp = P("stats_sb", 6)
    bcp = P("bc_sb", 2)
    ltp = P("lt", 2)
    lsgp = P("lsg", 2)
    mm_ps = P("mm_ps", 6, "PSUM")
    st_ps = P("st_ps", 2, "PSUM")

    # ---------------- resident weights / params ----------------
    wall = const.tile([128, WTOT], MF, name="wall")
    pars = const.tile([128, PTOT], F32, name="pars")
    ones_col = const.tile([128, 1], MF, name="ones_col")
    nc.gpsimd.memset(ones_col[:], 1.0)

    def preload_weights():
        sd = nc.scalar.dma_start
        sd(pars[:], io["pars"])
        sd(wall[:, :WHP], io["wall"][:, :WHP])          # w_lp (tiny)
        for s in range(3):                              # w_hp per stream
            a, b = WHP + s * 8 * 512, WHP + (s + 1) * 8 * 512
            sd(wall[:, a:b], io["wall"][:, a:b])
        sd(wall[:, WR:], io["wall"][:, WR:])            # all the rest

    def wch(seg, c, m):
        base = seg + c * 512 + m * 128
        return wall[:, base: base + 128]

    def wsrcs(seg, nk, rhs_fn):
        return [(lambda m, c=c: wch(seg, c, m), rhs_fn(c))
                for c in range(nk)]

    def pcol(k):
        return pars[:, k:k + 1]

    def pblk(k, n):
        return pars[:, k:k + n]

    # ---------------- helpers ----------------
    def emit_istd(v_sb):
        """v_sb: [1,512] sbuf fp32 variance (+eps already added).
        Returns a [1,512] MF istd row via DVE approx-reciprocal + ACT sqrt
        (rsqrt(v) = sqrt(1/v)); ~18-bit accurate, no PE transposes."""
        r = stp.tile([1, 512], F32, name="recip", tag="ssb")
        nc.vector.reciprocal_approx_fast(out=r[:], in_=v_sb[0:1, :])
        istd = stp.tile([1, 512], MF, name="istd", tag="ssb")
        nc.scalar.activation(istd[:], r[:], ACT.Sqrt)
        return istd

    def bcast(row_ap):
        """[1,512] sbuf row -> [128,512] sbuf tile via GPSIMD."""
        bc = bcp.tile([128, 512], MF, name="bc")
        nc.gpsimd.partition_broadcast(bc[:], row_ap)
        return bc

    def emit_ln(ps_list, bias_cols=None):
        """Evict psum chunks to SBUF (adding the layer bias per partition)
        and accumulate sum(y^2) into a [1,512] psum row."""
        nch = len(ps_list)
        ev = []
        for c, ps in enumerate(ps_list):
            e = evp.tile([128, 512], MF, name="ev")
            em.copy(e[:], ps[:], None if bias_cols is None else bias_cols[c])
            ev.append(e)
        st = st_ps.tile([1, 512], F32, name="st", tag="stat_ps")
        for c in range(nch):
            sq = sqp.tile([128, 512], MF, name="sq")
            em.square(sq[:], ev[c][:], ev[c][:])
            nc.tensor.matmul(st[:], ones_col[:], sq[:],
                             start=(c == 0), stop=(c == nch - 1))
        return ev, st

    def ln_finish(ev, st, gam, bet, gelu, out_tile, dim=D):
        v = stp.tile([1, 512], F32, name="v", tag="ssb")
        nc.vector.tensor_scalar(v[0:1, :], st[:], 1.0 / dim, EPS,
                                ALU.mult, ALU.add)
        istd = emit_istd(v)
        bc = bcast(istd[:])
        func = ACT.Gelu if gelu else ACT.Identity
        for c, e in enumerate(ev):
            z = zp.tile([128, 512], MF, name="z")
            nc.vector.tensor_mul(z[:], e[:], bc[:])
            nc.scalar.activation(out_tile[:, c, :], z[:], func,
                                 bias=bet[:, c:c + 1],
                                 scale=gam[:, c:c + 1])

    def mm_group(n_m, srcs):
        """Emit an accumulating matmul group. srcs = list of (lhsT_fn, rhs)
        k-chunks; returns the psum tiles."""
        ps_list = [mm_ps.tile([128, 512], F32, name="mm") for _ in range(n_m)]
        last = len(srcs) - 1
        for ci, (lhsT_fn, rhs) in enumerate(srcs):
            for m in range(n_m):
                nc.tensor.matmul(ps_list[m][:], lhsT_fn(m), rhs,
                                 start=(ci == 0), stop=(ci == last))
        return ps_list

    # ---------------- per-bt state ----------------
    S = [dict(lt=[None] * 3, xc=[None] * 3, l=[None] * 3, yh=[None] * 3,
              e=[None] * 3, m=[None] * 3, zt=[None] * 3, wr=[None] * 3,
              h=None) for _ in range(NBT)]
    pend = []

    def flush(n=None):
        cnt = len(pend) if n is None else n
        for _ in range(cnt):
            if pend:
                pend.pop(0)()

    def emit_x_loads(bt):
        bsl = ts(bt, 512)
        for s in range(3):
            xc = xpool.tile([128, 8, 512], MF, name="xc")
            nc.sync.dma_start(xc[:], io[f"xT{s}"][:, bt])
            S[bt]["xc"][s] = xc
        for s in range(3):
            lt = ltp.tile([FS[s], 512], F32, name="lt")
            nc.sync.dma_start(lt[:], io[f"lT{s}"][:, bsl])
            S[bt]["lt"][s] = lt

    # ---------------- phase emitters ----------------
    def emit_lp_mm(bt, s):
        lsg = lsgp.tile([FS[s], 512], MF, name="lsg")
        nc.scalar.activation(lsg[:], S[bt]["lt"][s][:], ACT.Sigmoid)
        ps_list = [mm_ps.tile([128, 512], F32, name="mm") for _ in range(2)]
        for m in range(2):
            nc.tensor.matmul(ps_list[m][:],
                             wall[0:FS[s], WLP + s * 256 + m * 128:
                                  WLP + s * 256 + (m + 1) * 128],
                             lsg[:], start=True, stop=True)
        ev, st = emit_ln(ps_list, [pcol(PB_LP + s * 2 + c) for c in range(2)])

        def fin(s=s, bt=bt, ev=ev, st=st):
            l_sb = lp_.tile([128, 2, 512], MF, name="l_sb")
            ln_finish(ev, st, pblk(PG_LP + s * 2, 2), pblk(PBE_LP + s * 2, 2),
                      True, l_sb, dim=D // 2)
            S[bt]["l"][s] = l_sb
        pend.append(fin)

    def emit_hp_mm(bt, s):
        xc = S[bt]["xc"][s]
        srcs = [(lambda m, c=c: wch(WHP, s * 8 + c, m), xc[:, c, :])
                for c in range(8)]
        ps_list = mm_group(4, srcs)
        ev, st = emit_ln(ps_list, [pcol(PB_HP + s * 4 + c) for c in range(4)])

        def fin(s=s, bt=bt, ev=ev, st=st):
            yh = yhp.tile([128, 4, 512], MF, name="yh")
            ln_finish(ev, st, pblk(PG_HP + s * 4, 4), pblk(PBE_HP + s * 4, 4),
                      True, yh)
            S[bt]["yh"][s] = yh
        pend.append(fin)

    def emit_r_mm(bt, s):
        yh = S[bt]["yh"][s]
        srcs = wsrcs(WR, 4, lambda c: yh[:, c, :])
        ps_list = mm_group(4, srcs)
        ev, st = emit_ln(ps_list, [pcol(PB_R + c) for c in range(4)])

        def fin(s=s, bt=bt, ev=ev, st=st):
            e_sb = ep.tile([128, 4, 512], MF, name="e_sb")
            ln_finish(ev, st, pblk(PG_N1 + s * 4, 4), pblk(PBE_N1 + s * 4, 4),
                      False, e_sb)
            S[bt]["e"][s] = e_sb
        pend.append(fin)

    m_streams = [(1, 2), (0, 2), (0, 1)]

    def emit_m_mm(bt, s):
        sa, sb = m_streams[s]
        e_tiles = S[bt]["e"]
        srcs = [(lambda m, c=c: wch(WM, s * 8 + c, m),
                 (e_tiles[sa][:, c, :] if c < 4 else e_tiles[sb][:, c - 4, :]))
                for c in range(8)]
        ps_list = mm_group(4, srcs)
        m_sb = mp.tile([128, 4, 512], MF, name="m_sb")
        for c in range(4):
            em.copy(m_sb[:, c, :], ps_list[c][:], pcol(PB_M + s * 4 + c))
        S[bt]["m"][s] = m_sb

    def emit_gate_fuse(bt, s):
        e_sb, m_sb = S[bt]["e"][s], S[bt]["m"][s]
        srcs = [(lambda m, c=c: wch(WG, s * 8 + c, m),
                 (e_sb[:, c, :] if c < 4 else m_sb[:, c - 4, :]))
                for c in range(8)]
        ps_list = mm_group(4, srcs)
        t_sb = tp.tile([128, 4, 512], MF, name="t_sb")
        for c in range(4):
            sg = sgp.tile([128, 512], MF, name="sg")
            nc.scalar.activation(sg[:], ps_list[c][:], ACT.Sigmoid,
                                 bias=pcol(PB_G + s * 4 + c))
            q = qp.tile([128, 512], MF, name="q")
            nc.vector.tensor_mul(q[:], sg[:], m_sb[:, c, :])
            nc.vector.tensor_add(t_sb[:, c, :], e_sb[:, c, :], q[:])
        st_sum = st_ps.tile([1, 512], F32, name="st_sum", tag="stat_ps")
        st_sq = st_ps.tile([1, 512], F32, name="st_sq", tag="stat_ps")
        for c in range(4):
            nc.tensor.matmul(st_sum[:], ones_col[:], t_sb[:, c, :],
                             start=(c == 0), stop=(c == 3))
        for c in range(4):
            sq = sqp.tile([128, 512], MF, name="sq")
            em.square(sq[:], t_sb[:, c, :], t_sb[:, c, :])
            nc.tensor.matmul(st_sq[:], ones_col[:], sq[:],
                             start=(c == 0), stop=(c == 3))

        def fin(s=s, bt=bt, t_sb=t_sb, st_sum=st_sum, st_sq=st_sq):
            mu = stp.tile([1, 512], F32, name="mu", tag="ssb")
            nc.vector.tensor_scalar_mul(mu[:], st_sum[:], 1.0 / D)
            ev2 = stp.tile([1, 512], F32, name="ev2", tag="ssb")
            nc.vector.tensor_scalar(ev2[:], st_sq[:], 1.0 / D, EPS,
                                    ALU.mult, ALU.add)
            v = stp.tile([1, 512], F32, name="v", tag="ssb")
            nc.vector.tensor_mul(v[:], mu[:], mu[:])
            nc.vector.tensor_sub(v[:], ev2[:], v[:])
            istd = emit_istd(v)
            w_row = stp.tile([1, 512], MF, name="w_row", tag="ssb")
            nc.vector.tensor_mul(w_row[:], mu[:], istd[:])
            S[bt]["wr"][s] = w_row
            bc = bcast(istd[:])
            for c in range(4):
                nc.vector.tensor_mul(t_sb[:, c, :], t_sb[:, c, :], bc[:])
            S[bt]["zt"][s] = t_sb
        pend.append(fin)

    def emit_f1(bt):
        # order k-chunks so zt0 (finished last) is consumed last
        srcs = []
        for s in range(3):
            srcs += [(lambda m, c=c, s=s: wch(WF1L, s * 2 + c, m),
                      S[bt]["l"][s][:, c, :]) for c in range(2)]
        for s in (2, 1, 0):
            srcs += [(lambda m, c=c, s=s: wch(WF1, s * 4 + c, m),
                      S[bt]["zt"][s][:, c, :]) for c in range(4)]
        for s in (2, 1, 0):
            srcs.append((lambda m, s=s: wall[0:1, NEGC + s * 512 + m * 128:
                                             NEGC + s * 512 + (m + 1) * 128],
                         S[bt]["wr"][s][:]))
        ps_list = mm_group(4, srcs)
        ev, st = emit_ln(ps_list, [pcol(PB_F1 + c) for c in range(4)])

        def fin(bt=bt, ev=ev, st=st):
            h_sb = hp_.tile([128, 4, 512], MF, name="h_sb")
            ln_finish(ev, st, pblk(PG_F1, 4), pblk(PBE_F1, 4), True, h_sb)
            S[bt]["h"] = h_sb
        pend.append(fin)

    def emit_f2(bt):
        h_sb = S[bt]["h"]
        srcs = wsrcs(WF2, 4, lambda c: h_sb[:, c, :])
        ps_list = mm_group(4, srcs)
        ev, st = emit_ln(ps_list, [pcol(PB_F2 + c) for c in range(4)])

        def fin(bt=bt, ev=ev, st=st):
            bsl = ts(bt, 512)
            o_sb = op_.tile([128, 4, 512], MF, name="o_sb")
            ln_finish(ev, st, pblk(PG_F2, 4), pblk(PBE_F2, 4), False, o_sb)
            nc.sync.dma_start(io["outT"][:, bt], o_sb[:])
        pend.append(fin)

    # ---------------- schedule (software pipelined across both bts) -----
    emit_x_loads(0)
    preload_weights()

    def mid_phases(bt):
        emit_r_mm(bt, 0)
        flush(1)               # hp1 fin
        emit_hp_mm(bt, 2)
        emit_r_mm(bt, 1)
        flush(1)               # n1_0 fin -> e0
        flush(1)               # hp2 fin -> yh2
        emit_r_mm(bt, 2)
        flush(1)               # n1_1 fin -> e1
        emit_m_mm(bt, 2)       # m_target needs e0,e1
        flush(1)               # n1_2 fin -> e2
        emit_gate_fuse(bt, 2)
        emit_m_mm(bt, 1)       # m_inst needs e0,e2
        flush(1)               # n2_2 fin -> zt2 (covered by m1 matmuls)
        emit_gate_fuse(bt, 1)
        emit_m_mm(bt, 0)       # m_verb needs e1,e2
        flush(1)               # n2_1 fin -> zt1 (covered by m0 matmuls)
        emit_gate_fuse(bt, 0)
        flush(1)               # n2_0 fin -> zt0 (covered by f1 l/zt2/zt1)

    # ---- bt 0 ----
    for s in range(3):
        emit_lp_mm(0, s)
    emit_hp_mm(0, 0)           # hp0 matmuls cover lp fin chains
    flush(2)                   # lp0, lp1 fins
    emit_hp_mm(0, 1)
    flush(2)                   # lp2 fin + hp0 fin
    mid_phases(0)
    emit_x_loads(1)            # prefetch bt1 inputs
    emit_f1(0)
    # ---- bt1 head overlaps bt0 f1/f2 LN tails ----
    for s in range(3):
        emit_lp_mm(1, s)
    emit_hp_mm(1, 0)
    flush(1)                   # f1(0) fin -> h (covered by bt1 lp/hp mms)
    emit_f2(0)
    emit_hp_mm(1, 1)
    flush(2)                   # lp0(1), lp1(1) fins
    flush(2)                   # lp2(1), hp0(1) fins
    emit_r_mm(1, 0)
    flush(1)                   # f2(0) fin + output DMA (covered by r/hp mms)
    # ---- bt 1 remainder (mirrors bt0 mid/tail) ----
    flush(1)                   # hp1(1) fin
    emit_hp_mm(1, 2)
    emit_r_mm(1, 1)
    flush(1)                   # e0
    flush(1)                   # yh2
    emit_r_mm(1, 2)
    flush(1)                   # e1
    emit_m_mm(1, 2)
    flush(1)                   # e2
    emit_gate_fuse(1, 2)
    emit_m_mm(1, 1)
    flush(1)                   # zt2
    emit_gate_fuse(1, 1)
    emit_m_mm(1, 0)
    flush(1)                   # zt1
    emit_gate_fuse(1, 0)
    flush(1)                   # zt0
    emit_f1(1)
    flush(1)                   # f1(1) fin
    emit_f2(1)
    flush()
    ctx.close()


def build_program():
    nc = bacc.Bacc("TRN2", target_bir_lowering=False, debug=False,
                   num_devices=NCORES)
    io = {}

    def din(name, shape, dtype=F32):
        io[name] = nc.dram_tensor(name, list(shape), dtype,
                                  kind="ExternalInput").ap()

    for s in range(3):
        din(f"xT{s}", (128, NBT, 8, 512), dtype=MF)
        din(f"lT{s}", (FS[s], BL))
    din("wall", (128, WTOT), dtype=MF)
    din("pars", (128, PTOT))
    io["outT"] = nc.dram_tensor("outT", [128, NBT, 4, 512], MF,
                                kind="ExternalOutput").ap()

    with tile.TileContext(nc) as tc:
        emit_program(tc, io)
    nc.compile()
    return nc


def make_in_maps(inputs):
    fw = fold_weights(inputs)
    wall = pack_wall(fw)
    pars = pack_pars(fw)
    # device layout: x[p, bt, c, j] = xT[c*128+p, bt*512+j] per core slice
    hidden = [np.asarray(inputs["verb_hidden"], np.float32).T.astype(MM_NP),
              np.asarray(inputs["inst_hidden"], np.float32).T.astype(MM_NP),
              np.asarray(inputs["target_hidden"], np.float32).T.astype(MM_NP)]
    logits = [np.asarray(inputs["verb_logits"], np.float32),
              np.asarray(inputs["inst_logits"], np.float32),
              np.asarray(inputs["target_logits"], np.float32)]
    in_maps = []
    for core in range(NCORES):
        rows = slice(core * BL, (core + 1) * BL)
        m = {"wall": wall, "pars": pars}
        for s in range(3):
            xc = hidden[s][:, rows].reshape(8, 128, NBT, 512)
            m[f"xT{s}"] = np.ascontiguousarray(xc.transpose(1, 2, 0, 3))
            m[f"lT{s}"] = np.ascontiguousarray(logits[s][rows].T)
        in_maps.append(m)
    return in_maps


_NC_CACHE = None


def _run(inputs, **spmd_kwargs):
    global _NC_CACHE
    if _NC_CACHE is None:
        _NC_CACHE = build_program()
    nc = _NC_CACHE
    in_maps = make_in_maps(inputs)
    res = run_bass_kernel_spmd(nc, in_maps, list(range(NCORES)),
                               **spmd_kwargs)
    out = np.empty((B, D), dtype=np.float32)
    for core in range(NCORES):
        o = res.results[core]["outT"]          # [128, NBT, 4, 512]
        out[core * BL:(core + 1) * BL] = \
            o.transpose(1, 3, 2, 0).reshape(BL, D).astype(np.float32)
    return out, res


def kernel(**inputs) -> np.ndarray:
    return _run(inputs)[0]


def kernel_profiled(inputs, tmpdir=None):
    """Returns (out, BassKernelResults) with an NTFF-based profile."""
    return _run(inputs, trace=True, tmpdir=tmpdir)
